# revision 39
# baseline (speedup 1.0000x reference)
"""Trainium2 Bass kernel for nn_CZT_prop: chirp-z (Bluestein) optical propagation.

Math: per wavelength both CZT axes share the transform M = diag(q) Tc diag(p)
with Tc[c,k] = tau(c-k), tau(d) = exp(-i*aw*d^2/2) an EVEN Toeplitz kernel, so
Tc is centrosymmetric and block-diagonalizes: Tc = K blockdiag(T+, T-) K / 2,
T+/-[c,k] = tau(c-k) +/- tau(c+k-1023), size 512.  The 2D result
    out = F0' . (Tc V Tc^T),   V = (field.F) * outer(p,p)
splits into four half-size quadrant products G_ab = T_a Vt_ab T_b (a,b in {+,-})
where Vt_ab are the +/- fold combos of V (host-prepared).  This HALVES the
device MACs vs the dense formulation.  The recombine (16 flip-adds), the F0'
multiply, and a rank-1 correction for the reference's zero-padded corner entry
[1023,0] of Tc are all host-side assembly.

Device per core (= one (wavelength, a-class)): two chained Karatsuba complex
matmul stages, contraction 512:
    S_b[j,c] = sum_k Vt_ab[k,j] T_a[k,c]     (b in {self, other})
    G_ab[c,d] = sum_j S_b[j,c] T_b[j,d]
192 fp16 matmuls of [128x512]@K=128 = 98304 PE cycles.  Sharding: 8 cores =
4 wavelengths x 2 centro-classes.  Zero communication.
"""
import math
import numpy as np

f32 = np.float32
f16 = np.float16
f64 = np.float64

# ---- static geometry (mirrors the problem spec) ----
H = 1024
M = 1024
N_WL = 4
DX = 100e-6
ODX = 10e-6
ODY = 10e-6
Z = 0.05
TWO_PI = 2.0 * np.pi
P = 128
HN = 512
NT = 4          # 128-row tiles per 512
X_IN = np.linspace(-H * DX / 2, H * DX / 2, H).astype(f64)
X_OUT = np.linspace(-M * ODX / 2, M * ODX / 2, M).astype(f64)


def _pow2_below(x):
    return 2.0 ** math.floor(math.log2(x))


def czt_factors(wl):
    """q[c], p[k], aw for the 1-axis CZT: out[c] = q[c] sum_k tau(c-k) p[k] x[k]."""
    Dm = wl * Z / DX
    f1 = X_OUT[0] + Dm / 2
    f2 = X_OUT[-1] + Dm / 2
    D1 = f1 + (M * Dm + f2 - f1) / (2 * M)
    D2 = f2 + (M * Dm + f2 - f1) / (2 * M)
    alpha_A = TWO_PI * D1 / Dm
    aw = -TWO_PI * (D1 - D2) / (M * Dm)
    k = np.arange(H, dtype=f64)
    c = np.arange(M, dtype=f64)
    h = lambda x: np.exp(1j * aw * x * x / 2)
    l = c / M * (D2 - D1) + D1
    m_shift = np.exp(-1j * TWO_PI * l * (-H / 2 + 0.5) / Dm)
    q = m_shift * h(c) * np.exp(-1j * aw * c) * np.exp(-1j * aw / 2)
    p = np.exp(-1j * alpha_A * k) * h(k) * np.exp(1j * aw * k)
    return q, p, aw


def _rs_kernel_full(xg, wl):
    """RS transfer kernel on the full plane via its 4-fold quad symmetry."""
    kv = TWO_PI / wl
    xh = xg[:HN]
    r2 = xh[:, None] ** 2 + xh[None, :] ** 2 + Z * Z
    r = np.sqrt(r2)
    aa = (Z / TWO_PI) / (r2 * r)
    bt = (kv * Z / TWO_PI) / r2
    ph = kv * r
    cq, sq = np.cos(ph), np.sin(ph)
    Fq = (aa * cq + bt * sq) + 1j * (aa * sq - bt * cq)
    return np.block([[Fq, Fq[:, ::-1]], [Fq[::-1, :], Fq[::-1, ::-1]]])


def host_prepare(field_real, field_imag, wavelengths):
    """Per-core device inputs + assembly metadata.  All f64 host math."""
    wls = np.asarray(wavelengths, f64)
    Jr = np.arange(HN)[::-1]
    in_maps = [None] * 8
    meta = []
    for w in range(N_WL):
        wl = f64(wls[w])
        q, p, aw = czt_factors(wl)
        tau = lambda d: np.exp(-1j * aw * np.asarray(d, f64) ** 2 / 2)
        F = _rs_kernel_full(X_IN, wl)
        F0 = _rs_kernel_full(X_OUT, wl)
        U = (np.asarray(field_real[0, w], f64)
             + 1j * np.asarray(field_imag[0, w], f64)) * F
        V = U * np.outer(p, p)

        cc = np.arange(HN, dtype=f64)[:, None]
        kk = np.arange(HN, dtype=f64)[None, :]
        tnear = tau(cc - kk)
        tfar = tau(cc + kk - (H - 1))
        Tp = tnear + tfar
        Tm = tnear - tfar

        V11 = V[:HN, :HN]; V12 = V[:HN, HN:]
        V21 = V[HN:, :HN]; V22 = V[HN:, HN:]
        A12 = V12[:, Jr]; A21 = V21[Jr, :]; A22 = V22[Jr][:, Jr]
        Vt = {('+', '+'): (V11 + A12 + A21 + A22) / 4,
              ('+', '-'): (V11 - A12 + A21 - A22) / 4,
              ('-', '+'): (V11 + A12 - A21 - A22) / 4,
              ('-', '-'): (V11 - A12 - A21 + A22) / 4}

        # pow2 scaling: one scale per (a,b) block, chosen so BOTH stage
        # outputs stay in fp16 range with ~8x headroom (stage-2 dominates).
        mt2 = {'+': float(np.mean(np.abs(Tp) ** 2)),
               '-': float(np.mean(np.abs(Tm) ** 2))}
        scales = {}
        for ab, Vab in Vt.items():
            fro2 = float(np.sum(np.abs(Vab) ** 2))
            s2_raw = 8.0 * math.sqrt(0.5 * mt2[ab[0]] * mt2[ab[1]] * fro2)
            scales[ab] = _pow2_below(8192.0 / max(s2_raw, 1e-300))

        def detile(z):
            """complex [512,512] -> [128, 4096]: i-plane kt-blocks then r."""
            zi = f16(z.imag); zr = f16(z.real)
            blocks = [zi[P * t:P * (t + 1), :] for t in range(NT)] \
                   + [zr[P * t:P * (t + 1), :] for t in range(NT)]
            return np.ascontiguousarray(np.concatenate(blocks, axis=1))

        Tdev = {'+': detile(Tp), '-': detile(Tm)}
        for ai, a in enumerate(('+', '-')):
            b_self, b_oth = a, ('-' if a == '+' else '+')
            vs = Vt[(a, b_self)] * scales[(a, b_self)]
            vo = Vt[(a, b_oth)] * scales[(a, b_oth)]
            in_maps[2 * w + ai] = {
                "ts": Tdev[a],
                "to": Tdev[b_oth],
                "vs": detile(vs),
                "vo": detile(vo),
            }

        # rank-1 corner correction (f64, exact): out = Tc_ref V Tc_ref^T,
        # Tc_ref = Tc - s e_1023 e_0^T.  Row/col vectors via T+/- blocks:
        # T11 = (Tp+Tm)/2, H = (Tp-Tm)/2 (T12 = H J, T21 = J H, T22 = J T11 J)
        s = tau(f64(H - 1))
        T11 = (Tp + Tm) / 2
        Hh = (Tp - Tm) / 2
        v1 = V[0, :HN]; v2r = V[0, HN:][Jr]
        row = np.empty(M, np.complex128)
        row[:HN] = v1 @ T11 + v2r @ Hh
        row[HN:] = (v1 @ Hh + v2r @ T11)[Jr]
        u1 = V[:HN, 0]; u2r = V[HN:, 0][Jr]
        col = np.empty(M, np.complex128)
        col[:HN] = T11 @ u1 + Hh @ u2r
        col[HN:] = (Hh @ u1 + T11 @ u2r)[Jr]
        # F0' with all diagonal factors + global scale folded
        F0p = F0 * np.outer(q, q) * (Z * ODX * ODY * wl)
        meta.append({
            "w": w,
            "scales": scales,
            "F0p": F0p.astype(np.complex128),
            "corr_row": s * row, "corr_col": s * col,
            "corr_s": s * s * V[0, 0],
        })
    return in_maps, meta


def assemble(results, meta):
    out = np.zeros((1, N_WL, M, M), np.complex64)
    Jr = np.arange(HN)[::-1]
    for md in meta:
        w = md["w"]
        qs = {}
        for ai, a in enumerate(('+', '-')):
            r = results[2 * w + ai]
            b_self, b_oth = a, ('-' if a == '+' else '+')
            for key, ab in (("gs", (a, b_self)), ("go", (a, b_oth))):
                g = r[key].astype(f32)
                qs[ab] = (g[:, :HN].astype(f64) + 1j * g[:, HN:].astype(f64)) \
                    / md["scales"][ab]
        Q1 = qs[('+', '+')]; Q2 = qs[('+', '-')]
        Q3 = qs[('-', '+')]; Q4 = qs[('-', '-')]
        Gf = np.empty((M, M), np.complex128)
        Gf[:HN, :HN] = Q1 + Q2 + Q3 + Q4
        Gf[:HN, HN:] = (Q1 - Q2 + Q3 - Q4)[:, Jr]
        Gf[HN:, :HN] = (Q1 + Q2 - Q3 - Q4)[Jr, :]
        Gf[HN:, HN:] = (Q1 - Q2 - Q3 + Q4)[Jr][:, Jr]
        Gf[M - 1, :] -= md["corr_row"]
        Gf[:, M - 1] -= md["corr_col"]
        Gf[M - 1, M - 1] += md["corr_s"]
        out[0, w] = (md["F0p"] * Gf).astype(np.complex64)
    return out


# ---------------- golden (numpy) model of the device program ----------------

def golden_core(inp):
    def split(x):
        zi = np.vstack([x[:, HN * t:HN * (t + 1)] for t in range(NT)])
        zr = np.vstack([x[:, HN * (NT + t):HN * (NT + t + 1)] for t in range(NT)])
        return zr, zi

    def karatsuba(Ar, Ai, As, Br, Bi, Bs):
        P1 = Ar.astype(f32).T @ Br.astype(f32)
        P2 = Ai.astype(f32).T @ Bi.astype(f32)
        P3 = As.astype(f32).T @ Bs.astype(f32)
        Xr = f16(P1 - P2)
        Xi = f16(P3 - f32(P1 + P2))
        return Xr, Xi

    tsr, tsi = split(inp["ts"]); tss = f16(tsr.astype(f32) + tsi.astype(f32))
    tor_, toi = split(inp["to"]); tos = f16(tor_.astype(f32) + toi.astype(f32))
    out = {}
    for key, vkey, (br, bi, bs) in (("gs", "vs", (tsr, tsi, tss)),
                                    ("go", "vo", (tor_, toi, tos))):
        vr, vi = split(inp[vkey]); vv = f16(vr.astype(f32) + vi.astype(f32))
        Sr, Si = karatsuba(vr, vi, vv, tsr, tsi, tss)
        Ss = f16(Sr.astype(f32) + Si.astype(f32))
        Gr, Gi = karatsuba(Sr, Si, Ss, br, bi, bs)
        out[key] = np.concatenate([Gr, Gi], axis=1)
    return out


def golden(field_real, field_imag, wavelengths):
    in_maps, meta = host_prepare(field_real, field_imag, wavelengths)
    results = [golden_core(m) for m in in_maps]
    return assemble(results, meta)


# ---------------- bass program ----------------

_PROGRAM = None


def build_program():
    import concourse.bass as bass
    import concourse.tile as tile
    import concourse.mybir as mybir
    from concourse import bacc

    dt = mybir.dt
    ALU = mybir.AluOpType

    nc = bacc.Bacc("TRN2", target_bir_lowering=False, debug=False, num_devices=8)

    ts_d = nc.dram_tensor("ts", [P, 4096], dt.float16, kind="ExternalInput").ap()
    to_d = nc.dram_tensor("to", [P, 4096], dt.float16, kind="ExternalInput").ap()
    vs_d = nc.dram_tensor("vs", [P, 4096], dt.float16, kind="ExternalInput").ap()
    vo_d = nc.dram_tensor("vo", [P, 4096], dt.float16, kind="ExternalInput").ap()
    gs_d = nc.dram_tensor("gs", [HN, 1024], dt.float16, kind="ExternalOutput").ap()
    go_d = nc.dram_tensor("go", [HN, 1024], dt.float16, kind="ExternalOutput").ap()

    B2 = NT * HN  # 2048

    with tile.TileContext(nc) as tc:
      with tc.tile_pool(name="persist", bufs=1) as pp, \
           tc.tile_pool(name="psum", bufs=1, space="PSUM") as pspool, \
           tc.tile_pool(name="tmp", bufs=4) as tp:

        def bigtile(nm):
            return pp.tile([P, 4096], dt.float16, tag=nm, name=nm)
        TSa, TOa, VSa, VOa = bigtile("TSa"), bigtile("TOa"), bigtile("VSa"), bigtile("VOa")

        def plane_aps(big):
            iA = [big[:, HN * t:HN * (t + 1)] for t in range(NT)]
            rA = [big[:, B2 + HN * t:B2 + HN * (t + 1)] for t in range(NT)]
            return rA, iA
        TSrA, TSiA = plane_aps(TSa)
        TOrA, TOiA = plane_aps(TOa)
        VSrA, VSiA = plane_aps(VSa)
        VOrA, VOiA = plane_aps(VOa)

        def planes(nm):
            return [pp.tile([P, HN], dt.float16, tag=f"{nm}{t}", name=f"{nm}{t}")
                    for t in range(NT)]
        TSs, TOs, VSs, VOs = planes("TSs"), planes("TOs"), planes("VSs"), planes("VOs")
        Sr = {b: planes(f"S{b}r") for b in "so"}
        Si = {b: planes(f"S{b}i") for b in "so"}
        Ss = {b: planes(f"S{b}s") for b in "so"}

        wlhs = pp.tile([P, P], dt.float16, tag="wlhs", name="wlhs")
        wrhs = pp.tile([P, HN], dt.float16, tag="wrhs", name="wrhs")
        nc.vector.memset(wlhs[:], 0.0)
        nc.vector.memset(wrhs[:], 0.0)

        # ---- input DMAs in consumption order (P2s, P2o, P1s, P1o, P3s/P3o,
        # stage2-other) so matmul passes chase arrivals with no stalls ----
        # sync: tsi kt0-3, tsr kt0-3, vor kt0-3, to halves
        for t in range(NT):
            nc.sync.dma_start(TSiA[t], ts_d[:, HN * t:HN * (t + 1)])
        for t in range(NT):
            nc.sync.dma_start(TSrA[t], ts_d[:, B2 + HN * t:B2 + HN * (t + 1)])
        for t in range(NT):
            nc.sync.dma_start(VOrA[t], vo_d[:, B2 + HN * t:B2 + HN * (t + 1)])
        nc.sync.dma_start(TOa[:, 0:B2], to_d[:, 0:B2])
        nc.sync.dma_start(TOa[:, B2:4096], to_d[:, B2:4096])
        # gpsimd: vsi kt0-3, voi kt0-3, vsr kt0-3
        for t in range(NT):
            nc.gpsimd.dma_start(VSiA[t], vs_d[:, HN * t:HN * (t + 1)])
        for t in range(NT):
            nc.gpsimd.dma_start(VOiA[t], vo_d[:, HN * t:HN * (t + 1)])
        for t in range(NT):
            nc.gpsimd.dma_start(VSrA[t], vs_d[:, B2 + HN * t:B2 + HN * (t + 1)])

        _wn = [0]

        def warmup(n):
            for _ in range(n):
                i = _wn[0]
                _wn[0] += 1
                wp = pspool.tile([P, HN], dt.float32, tag=f"ps{6 + i % 2}", name=f"wps{i}")
                nc.tensor.matmul(wp[:], lhsT=wlhs[:], rhs=wrhs[:], start=True, stop=True)

        warmup(7)

        # sum planes: vector takes ts/vs (needed by P3s ~ mid-flight),
        # gpsimd takes vo/to (needed later)
        for t in range(NT):
            nc.vector.tensor_tensor(out=TSs[t][:], in0=TSrA[t], in1=TSiA[t], op=ALU.add)
            nc.vector.tensor_tensor(out=VSs[t][:], in0=VSrA[t], in1=VSiA[t], op=ALU.add)
        for t in range(NT):
            nc.gpsimd.tensor_tensor(out=VOs[t][:], in0=VOrA[t], in1=VOiA[t], op=ALU.add)
            nc.gpsimd.tensor_tensor(out=TOs[t][:], in0=TOrA[t][:],
                                    in1=TOiA[t][:], op=ALU.add)

        A = lambda ts_: [t[:] for t in ts_]
        TSsA, TOsA, VSsA, VOsA = A(TSs), A(TOs), A(VSs), A(VOs)
        SrA = {b: A(Sr[b]) for b in "so"}
        SiA = {b: A(Si[b]) for b in "so"}
        SsA = {b: A(Ss[b]) for b in "so"}

        # ---- stage 1, both b-blocks, stream-chasing kt-outer passes ----
        # banks: self-set mt -> bank mt, other-set mt -> bank 4+mt
        def acc_pass(ps, lhs, rhs):
            for kt in range(NT):
                st, sp = (kt == 0), (kt == NT - 1)
                for mt in range(NT):
                    msl = slice(P * mt, P * (mt + 1))
                    nc.tensor.matmul(ps[mt][:], lhsT=lhs[kt][:, msl], rhs=rhs[kt],
                                     start=st, stop=sp)

        p2S = [pspool.tile([P, HN], dt.float32, tag=f"ps{mt}", name=f"p2S{mt}")
               for mt in range(NT)]
        p2O = [pspool.tile([P, HN], dt.float32, tag=f"ps{4 + mt}", name=f"p2O{mt}")
               for mt in range(NT)]
        acc_pass(p2S, VSiA, TSiA)
        acc_pass(p2O, VOiA, TSiA)
        p2cS, p2cO = [], []
        for mt in range(NT):
            c = tp.tile([P, HN], dt.float32, tag="p2c", name=f"p2cS{mt}")
            nc.scalar.mul(c[:], p2S[mt][:], 1.0)
            p2cS.append(c)
        for mt in range(NT):
            c = tp.tile([P, HN], dt.float32, tag="p2co", name=f"p2cO{mt}")
            nc.scalar.mul(c[:], p2O[mt][:], 1.0)
            p2cO.append(c)
        p1S = [pspool.tile([P, HN], dt.float32, tag=f"ps{mt}", name=f"p1S{mt}")
               for mt in range(NT)]
        acc_pass(p1S, VSrA, TSrA)
        p1O = [pspool.tile([P, HN], dt.float32, tag=f"ps{4 + mt}", name=f"p1O{mt}")
               for mt in range(NT)]
        acc_pass(p1O, VOrA, TSrA)
        # Xr/t01 for self while P1o runs on the PE
        t01S, t01O = [], []
        for mt in range(NT):
            t01 = tp.tile([P, HN], dt.float32, tag="t01", name=f"t01S{mt}")
            nc.vector.tensor_tensor(out=SrA['s'][mt], in0=p1S[mt][:], in1=p2cS[mt][:],
                                    op=ALU.subtract)
            nc.vector.tensor_tensor(out=t01[:], in0=p1S[mt][:], in1=p2cS[mt][:], op=ALU.add)
            t01S.append(t01)
        # P3 self sweeps (bank mt), with per-mt combines trailing
        for mt in range(NT):
            msl = slice(P * mt, P * (mt + 1))
            p3 = pspool.tile([P, HN], dt.float32, tag=f"ps{mt}", name=f"p3S{mt}")
            for kt in range(NT):
                nc.tensor.matmul(p3[:], lhsT=VSsA[kt][:, msl], rhs=TSsA[kt],
                                 start=(kt == 0), stop=(kt == NT - 1))
            if mt == 0:
                for m2 in range(NT):
                    t01 = tp.tile([P, HN], dt.float32, tag="t01o", name=f"t01O{m2}")
                    nc.vector.tensor_tensor(out=SrA['o'][m2], in0=p1O[m2][:],
                                            in1=p2cO[m2][:], op=ALU.subtract)
                    nc.vector.tensor_tensor(out=t01[:], in0=p1O[m2][:], in1=p2cO[m2][:],
                                            op=ALU.add)
                    t01O.append(t01)
            nc.vector.tensor_tensor(out=SiA['s'][mt], in0=p3[:], in1=t01S[mt][:],
                                    op=ALU.subtract)
            nc.vector.tensor_tensor(out=SsA['s'][mt], in0=SrA['s'][mt], in1=SiA['s'][mt],
                                    op=ALU.add)
        for mt in range(NT):
            msl = slice(P * mt, P * (mt + 1))
            p3 = pspool.tile([P, HN], dt.float32, tag=f"ps{4 + mt}", name=f"p3O{mt}")
            for kt in range(NT):
                nc.tensor.matmul(p3[:], lhsT=VOsA[kt][:, msl], rhs=TSsA[kt],
                                 start=(kt == 0), stop=(kt == NT - 1))
            nc.vector.tensor_tensor(out=SiA['o'][mt], in0=p3[:], in1=t01O[mt][:],
                                    op=ALU.subtract)
            nc.vector.tensor_tensor(out=SsA['o'][mt], in0=SrA['o'][mt], in1=SiA['o'][mt],
                                    op=ALU.add)

        # ---- stage 2, same stream-chasing pass structure as stage 1 ----
        # banks: self mt -> mt, other mt -> 4+mt (freed by stage-1 combines)
        q2S = [pspool.tile([P, HN], dt.float32, tag=f"ps{mt}", name=f"q2S{mt}")
               for mt in range(NT)]
        acc_pass(q2S, SiA['s'], TSiA)
        q2O = [pspool.tile([P, HN], dt.float32, tag=f"ps{4 + mt}", name=f"q2O{mt}")
               for mt in range(NT)]
        acc_pass(q2O, SiA['o'], TOiA)
        q2cS, q2cO = [], []
        for mt in range(NT):
            c = tp.tile([P, HN], dt.float32, tag="p2c", name=f"q2cS{mt}")
            nc.scalar.mul(c[:], q2S[mt][:], 1.0)
            q2cS.append(c)
        for mt in range(NT):
            c = tp.tile([P, HN], dt.float32, tag="p2co", name=f"q2cO{mt}")
            nc.scalar.mul(c[:], q2O[mt][:], 1.0)
            q2cO.append(c)
        q1S = [pspool.tile([P, HN], dt.float32, tag=f"ps{mt}", name=f"q1S{mt}")
               for mt in range(NT)]
        acc_pass(q1S, SrA['s'], TSrA)
        q1O = [pspool.tile([P, HN], dt.float32, tag=f"ps{4 + mt}", name=f"q1O{mt}")
               for mt in range(NT)]
        acc_pass(q1O, SrA['o'], TOrA)
        # G output tiles; r-half combines run while P1o / P3 passes occupy PE
        # stage psums to SBUF on scalar (second psum-reader) so the combine
        # ops are all-SBUF (2x DVE mode, and gpsimd-eligible)
        q1cS, q1cO = [], []
        for mt in range(NT):
            c = tp.tile([P, HN], dt.float32, tag="q1c", name=f"q1cS{mt}")
            nc.scalar.mul(c[:], q1S[mt][:], 1.0)
            q1cS.append(c)
        for mt in range(NT):
            c = tp.tile([P, HN], dt.float32, tag="q1co", name=f"q1cO{mt}")
            nc.scalar.mul(c[:], q1O[mt][:], 1.0)
            q1cO.append(c)
        gts = [tp.tile([P, 1024], dt.float16, tag="gouts", name=f"gts{mt}")
               for mt in range(NT)]
        gto = [tp.tile([P, 1024], dt.float16, tag="gouto", name=f"gto{mt}")
               for mt in range(NT)]
        t01S2, t01O2 = [], []
        for mt in range(NT):
            t01 = tp.tile([P, HN], dt.float32, tag="t01", name=f"t01S2_{mt}")
            nc.gpsimd.tensor_tensor(out=t01[:], in0=q1cS[mt][:], in1=q2cS[mt][:],
                                    op=ALU.add)
            nc.vector.tensor_tensor(out=gts[mt][:, 0:HN], in0=q1cS[mt][:],
                                    in1=q2cS[mt][:], op=ALU.subtract)
            t01S2.append(t01)
        for mt in range(NT):
            t01 = tp.tile([P, HN], dt.float32, tag="t01o", name=f"t01O2_{mt}")
            nc.gpsimd.tensor_tensor(out=t01[:], in0=q1cO[mt][:], in1=q2cO[mt][:],
                                    op=ALU.add)
            nc.vector.tensor_tensor(out=gto[mt][:, 0:HN], in0=q1cO[mt][:],
                                    in1=q2cO[mt][:], op=ALU.subtract)
            t01O2.append(t01)
        for mt in range(NT):
            msl = slice(P * mt, P * (mt + 1))
            p3 = pspool.tile([P, HN], dt.float32, tag=f"ps{mt}", name=f"q3S{mt}")
            for kt in range(NT):
                nc.tensor.matmul(p3[:], lhsT=SsA['s'][kt][:, msl], rhs=TSsA[kt],
                                 start=(kt == 0), stop=(kt == NT - 1))
            p3c = tp.tile([P, HN], dt.float32, tag="p3c", name=f"p3cS{mt}")
            nc.scalar.mul(p3c[:], p3[:], 1.0)
            nc.vector.tensor_tensor(out=gts[mt][:, HN:1024], in0=p3c[:],
                                    in1=t01S2[mt][:], op=ALU.subtract)
            nc.sync.dma_start(gs_d[msl, :], gts[mt][:])
        for mt in range(NT):
            msl = slice(P * mt, P * (mt + 1))
            last = (mt == NT - 1)
            halves = ((0, HN),) if not last else ((0, HN // 2), (HN // 2, HN))
            if last:
                nc.sync.dma_start(go_d[msl, 0:HN], gto[mt][:, 0:HN])
            for hi, (c0, c1) in enumerate(halves):
                wdt = c1 - c0
                full = (wdt == HN)
                p3 = pspool.tile([P, wdt], dt.float32,
                                 tag=f"ps{4 + mt}" if hi == 0 else f"ps{(5 + mt) % 8}",
                                 name=f"q3O{mt}_{hi}")
                for kt in range(NT):
                    nc.tensor.matmul(p3[:], lhsT=SsA['o'][kt][:, msl],
                                     rhs=TOsA[kt] if full else TOsA[kt][:, c0:c1],
                                     start=(kt == 0), stop=(kt == NT - 1))
                p3c = tp.tile([P, wdt], dt.float32, tag="p3co", name=f"p3cO{mt}_{hi}")
                if last:
                    nc.vector.tensor_copy(p3c[:], p3[:])
                else:
                    nc.scalar.mul(p3c[:], p3[:], 1.0)
                nc.vector.tensor_tensor(
                    out=gto[mt][:, HN:1024] if full else gto[mt][:, HN + c0:HN + c1],
                    in0=p3c[:],
                    in1=t01O2[mt][:] if full else t01O2[mt][:, c0:c1],
                    op=ALU.subtract)
                if last:
                    nc.sync.dma_start(go_d[msl, HN + c0:HN + c1],
                                      gto[mt][:, HN + c0:HN + c1])
            if not last:
                nc.sync.dma_start(go_d[msl, :], gto[mt][:])

    nc.compile()
    return nc


def get_program():
    global _PROGRAM
    if _PROGRAM is None:
        _PROGRAM = build_program()
    return _PROGRAM


def kernel(field_real, field_imag, wavelengths):
    field_real = np.asarray(field_real)
    field_imag = np.asarray(field_imag)
    wavelengths = np.asarray(wavelengths)
    in_maps, meta = host_prepare(field_real, field_imag, wavelengths)
    from concourse.bass_utils import run_bass_kernel_spmd
    nc = get_program()
    res = run_bass_kernel_spmd(nc, in_maps, core_ids=list(range(8)))
    return assemble(res.results, meta)


if __name__ == "__main__":
    import jax
    import reference as ref
    cpu = jax.devices("cpu")[0]
    with jax.default_device(cpu):
        inputs = {k: np.asarray(v) for k, v in ref.setup_inputs().items()}
        expected = np.asarray(ref.reference(**{k: jax.device_put(v, cpu)
                                               for k, v in inputs.items()}))
    got = golden(np.asarray(inputs["field_real"]), np.asarray(inputs["field_imag"]),
                 np.asarray(inputs["wavelengths"]))
    err = np.abs(got - expected)
    print(f"golden absmax err {err.max():.4g} rel {err.max() / np.abs(expected).max():.4g}")


# revision 40
# speedup vs baseline: 1.1124x; 1.1124x over previous
"""Trainium2 Bass kernel for nn_CZT_prop: chirp-z (Bluestein) optical propagation.

Math: per wavelength both CZT axes share the transform M = diag(q) Tc diag(p)
with Tc[c,k] = tau(c-k), tau(d) = exp(-i*aw*d^2/2) an EVEN Toeplitz kernel, so
Tc is centrosymmetric and block-diagonalizes: Tc = K blockdiag(T+, T-) K / 2,
T+/-[c,k] = tau(c-k) +/- tau(c+k-1023), size 512.  The 2D result
    out = F0' . (Tc V Tc^T),   V = (field.F) * outer(p,p)
splits into four half-size quadrant products G_ab = T_a Vt_ab T_b (a,b in {+,-})
where Vt_ab are the +/- fold combos of V (host-prepared).  This HALVES the
device MACs vs the dense formulation.  The recombine (16 flip-adds), the F0'
multiply, and a rank-1 correction for the reference's zero-padded corner entry
[1023,0] of Tc are all host-side assembly.

Device per core (= one (wavelength, a-class)): two chained Karatsuba complex
matmul stages, contraction 512:
    S_b[j,c] = sum_k Vt_ab[k,j] T_a[k,c]     (b in {self, other})
    G_ab[c,d] = sum_j S_b[j,c] T_b[j,d]
192 fp16 matmuls of [128x512]@K=128 = 98304 PE cycles.  Sharding: 8 cores =
4 wavelengths x 2 centro-classes.  Zero communication.
"""
import math
import numpy as np

f32 = np.float32
f16 = np.float16
f64 = np.float64

# ---- static geometry (mirrors the problem spec) ----
H = 1024
M = 1024
N_WL = 4
DX = 100e-6
ODX = 10e-6
ODY = 10e-6
Z = 0.05
TWO_PI = 2.0 * np.pi
P = 128
HN = 512
NT = 4          # 128-row tiles per 512
X_IN = np.linspace(-H * DX / 2, H * DX / 2, H).astype(f64)
X_OUT = np.linspace(-M * ODX / 2, M * ODX / 2, M).astype(f64)


def _pow2_below(x):
    return 2.0 ** math.floor(math.log2(x))


def czt_factors(wl):
    """q[c], p[k], aw for the 1-axis CZT: out[c] = q[c] sum_k tau(c-k) p[k] x[k]."""
    Dm = wl * Z / DX
    f1 = X_OUT[0] + Dm / 2
    f2 = X_OUT[-1] + Dm / 2
    D1 = f1 + (M * Dm + f2 - f1) / (2 * M)
    D2 = f2 + (M * Dm + f2 - f1) / (2 * M)
    alpha_A = TWO_PI * D1 / Dm
    aw = -TWO_PI * (D1 - D2) / (M * Dm)
    k = np.arange(H, dtype=f64)
    c = np.arange(M, dtype=f64)
    h = lambda x: np.exp(1j * aw * x * x / 2)
    l = c / M * (D2 - D1) + D1
    m_shift = np.exp(-1j * TWO_PI * l * (-H / 2 + 0.5) / Dm)
    q = m_shift * h(c) * np.exp(-1j * aw * c) * np.exp(-1j * aw / 2)
    p = np.exp(-1j * alpha_A * k) * h(k) * np.exp(1j * aw * k)
    return q, p, aw


def _rs_kernel_full(xg, wl):
    """RS transfer kernel on the full plane via its 4-fold quad symmetry."""
    kv = TWO_PI / wl
    xh = xg[:HN]
    r2 = xh[:, None] ** 2 + xh[None, :] ** 2 + Z * Z
    r = np.sqrt(r2)
    aa = (Z / TWO_PI) / (r2 * r)
    bt = (kv * Z / TWO_PI) / r2
    ph = kv * r
    cq, sq = np.cos(ph), np.sin(ph)
    Fq = (aa * cq + bt * sq) + 1j * (aa * sq - bt * cq)
    return np.block([[Fq, Fq[:, ::-1]], [Fq[::-1, :], Fq[::-1, ::-1]]])


def host_prepare(field_real, field_imag, wavelengths):
    """Per-core device inputs + assembly metadata.  All f64 host math."""
    wls = np.asarray(wavelengths, f64)
    Jr = np.arange(HN)[::-1]
    in_maps = [None] * 8
    meta = []
    for w in range(N_WL):
        wl = f64(wls[w])
        q, p, aw = czt_factors(wl)
        tau = lambda d: np.exp(-1j * aw * np.asarray(d, f64) ** 2 / 2)
        F = _rs_kernel_full(X_IN, wl)
        F0 = _rs_kernel_full(X_OUT, wl)
        U = (np.asarray(field_real[0, w], f64)
             + 1j * np.asarray(field_imag[0, w], f64)) * F
        V = U * np.outer(p, p)

        cc = np.arange(HN, dtype=f64)[:, None]
        kk = np.arange(HN, dtype=f64)[None, :]
        tnear = tau(cc - kk)
        tfar = tau(cc + kk - (H - 1))
        Tp = tnear + tfar
        Tm = tnear - tfar

        V11 = V[:HN, :HN]; V12 = V[:HN, HN:]
        V21 = V[HN:, :HN]; V22 = V[HN:, HN:]
        A12 = V12[:, Jr]; A21 = V21[Jr, :]; A22 = V22[Jr][:, Jr]
        Vt = {('+', '+'): (V11 + A12 + A21 + A22) / 4,
              ('+', '-'): (V11 - A12 + A21 - A22) / 4,
              ('-', '+'): (V11 + A12 - A21 - A22) / 4,
              ('-', '-'): (V11 - A12 - A21 + A22) / 4}

        # pow2 scaling: one scale per (a,b) block, chosen so BOTH stage
        # outputs stay in fp16 range with ~8x headroom (stage-2 dominates).
        mt2 = {'+': float(np.mean(np.abs(Tp) ** 2)),
               '-': float(np.mean(np.abs(Tm) ** 2))}
        scales = {}
        for ab, Vab in Vt.items():
            fro2 = float(np.sum(np.abs(Vab) ** 2))
            s2_raw = 8.0 * math.sqrt(0.5 * mt2[ab[0]] * mt2[ab[1]] * fro2)
            scales[ab] = _pow2_below(8192.0 / max(s2_raw, 1e-300))

        def detile(z):
            """complex [512,512] -> [128, 4096]: i-plane kt-blocks then r."""
            zi = f16(z.imag); zr = f16(z.real)
            blocks = [zi[P * t:P * (t + 1), :] for t in range(NT)] \
                   + [zr[P * t:P * (t + 1), :] for t in range(NT)]
            return np.ascontiguousarray(np.concatenate(blocks, axis=1))

        Tdev = {'+': detile(Tp), '-': detile(Tm)}
        for ai, a in enumerate(('+', '-')):
            b_self, b_oth = a, ('-' if a == '+' else '+')
            vs = Vt[(a, b_self)] * scales[(a, b_self)]
            vo = Vt[(a, b_oth)] * scales[(a, b_oth)]
            in_maps[2 * w + ai] = {
                "ts": Tdev[a],
                "to": Tdev[b_oth],
                "vs": detile(vs),
                "vo": detile(vo),
            }

        # rank-1 corner correction (f64, exact): out = Tc_ref V Tc_ref^T,
        # Tc_ref = Tc - s e_1023 e_0^T.  Row/col vectors via T+/- blocks:
        # T11 = (Tp+Tm)/2, H = (Tp-Tm)/2 (T12 = H J, T21 = J H, T22 = J T11 J)
        s = tau(f64(H - 1))
        T11 = (Tp + Tm) / 2
        Hh = (Tp - Tm) / 2
        v1 = V[0, :HN]; v2r = V[0, HN:][Jr]
        row = np.empty(M, np.complex128)
        row[:HN] = v1 @ T11 + v2r @ Hh
        row[HN:] = (v1 @ Hh + v2r @ T11)[Jr]
        u1 = V[:HN, 0]; u2r = V[HN:, 0][Jr]
        col = np.empty(M, np.complex128)
        col[:HN] = T11 @ u1 + Hh @ u2r
        col[HN:] = (Hh @ u1 + T11 @ u2r)[Jr]
        # F0' with all diagonal factors + global scale folded
        F0p = F0 * np.outer(q, q) * (Z * ODX * ODY * wl)
        meta.append({
            "w": w,
            "scales": scales,
            "F0p": F0p.astype(np.complex128),
            "corr_row": s * row, "corr_col": s * col,
            "corr_s": s * s * V[0, 0],
        })
    return in_maps, meta


def assemble(results, meta):
    out = np.zeros((1, N_WL, M, M), np.complex64)
    Jr = np.arange(HN)[::-1]
    for md in meta:
        w = md["w"]
        qs = {}
        for ai, a in enumerate(('+', '-')):
            r = results[2 * w + ai]
            b_self, b_oth = a, ('-' if a == '+' else '+')
            for key, ab in (("gs", (a, b_self)), ("go", (a, b_oth))):
                g = r[key].astype(f32)
                qs[ab] = (g[:, :HN].astype(f64) + 1j * g[:, HN:].astype(f64)) \
                    / md["scales"][ab]
        Q1 = qs[('+', '+')]; Q2 = qs[('+', '-')]
        Q3 = qs[('-', '+')]; Q4 = qs[('-', '-')]
        Gf = np.empty((M, M), np.complex128)
        Gf[:HN, :HN] = Q1 + Q2 + Q3 + Q4
        Gf[:HN, HN:] = (Q1 - Q2 + Q3 - Q4)[:, Jr]
        Gf[HN:, :HN] = (Q1 + Q2 - Q3 - Q4)[Jr, :]
        Gf[HN:, HN:] = (Q1 - Q2 - Q3 + Q4)[Jr][:, Jr]
        Gf[M - 1, :] -= md["corr_row"]
        Gf[:, M - 1] -= md["corr_col"]
        Gf[M - 1, M - 1] += md["corr_s"]
        out[0, w] = (md["F0p"] * Gf).astype(np.complex64)
    return out


# ---------------- golden (numpy) model of the device program ----------------

def golden_core(inp):
    def split(x):
        zi = np.vstack([x[:, HN * t:HN * (t + 1)] for t in range(NT)])
        zr = np.vstack([x[:, HN * (NT + t):HN * (NT + t + 1)] for t in range(NT)])
        return zr, zi

    def karatsuba(Ar, Ai, As, Br, Bi, Bs):
        P1 = Ar.astype(f32).T @ Br.astype(f32)
        P2 = Ai.astype(f32).T @ Bi.astype(f32)
        P3 = As.astype(f32).T @ Bs.astype(f32)
        Xr = f16(P1 - P2)
        Xi = f16(P3 - f32(P1 + P2))
        return Xr, Xi

    tsr, tsi = split(inp["ts"]); tss = f16(tsr.astype(f32) + tsi.astype(f32))
    tor_, toi = split(inp["to"]); tos = f16(tor_.astype(f32) + toi.astype(f32))
    out = {}
    for key, vkey, (br, bi, bs) in (("gs", "vs", (tsr, tsi, tss)),
                                    ("go", "vo", (tor_, toi, tos))):
        vr, vi = split(inp[vkey]); vv = f16(vr.astype(f32) + vi.astype(f32))
        Sr, Si = karatsuba(vr, vi, vv, tsr, tsi, tss)
        Ss = f16(Sr.astype(f32) + Si.astype(f32))
        Gr, Gi = karatsuba(Sr, Si, Ss, br, bi, bs)
        out[key] = np.concatenate([Gr, Gi], axis=1)
    return out


def golden(field_real, field_imag, wavelengths):
    in_maps, meta = host_prepare(field_real, field_imag, wavelengths)
    results = [golden_core(m) for m in in_maps]
    return assemble(results, meta)


# ---------------- bass program ----------------

_PROGRAM = None


def build_program():
    import concourse.bass as bass
    import concourse.tile as tile
    import concourse.mybir as mybir
    from concourse import bacc

    dt = mybir.dt
    ALU = mybir.AluOpType

    nc = bacc.Bacc("TRN2", target_bir_lowering=False, debug=False, num_devices=8)

    ts_d = nc.dram_tensor("ts", [P, 4096], dt.float16, kind="ExternalInput").ap()
    to_d = nc.dram_tensor("to", [P, 4096], dt.float16, kind="ExternalInput").ap()
    vs_d = nc.dram_tensor("vs", [P, 4096], dt.float16, kind="ExternalInput").ap()
    vo_d = nc.dram_tensor("vo", [P, 4096], dt.float16, kind="ExternalInput").ap()
    gs_d = nc.dram_tensor("gs", [HN, 1024], dt.float16, kind="ExternalOutput").ap()
    go_d = nc.dram_tensor("go", [HN, 1024], dt.float16, kind="ExternalOutput").ap()

    B2 = NT * HN  # 2048

    with tile.TileContext(nc) as tc:
      with tc.tile_pool(name="persist", bufs=1) as pp, \
           tc.tile_pool(name="psum", bufs=1, space="PSUM") as pspool, \
           tc.tile_pool(name="tmp", bufs=4) as tp:

        def bigtile(nm):
            return pp.tile([P, 4096], dt.float16, tag=nm, name=nm)
        TSa, TOa, VSa, VOa = bigtile("TSa"), bigtile("TOa"), bigtile("VSa"), bigtile("VOa")

        def plane_aps(big):
            iA = [big[:, HN * t:HN * (t + 1)] for t in range(NT)]
            rA = [big[:, B2 + HN * t:B2 + HN * (t + 1)] for t in range(NT)]
            return rA, iA
        TSrA, TSiA = plane_aps(TSa)
        TOrA, TOiA = plane_aps(TOa)
        VSrA, VSiA = plane_aps(VSa)
        VOrA, VOiA = plane_aps(VOa)

        def planes(nm):
            return [pp.tile([P, HN], dt.float16, tag=f"{nm}{t}", name=f"{nm}{t}")
                    for t in range(NT)]
        TSs, TOs, VSs, VOs = planes("TSs"), planes("TOs"), planes("VSs"), planes("VOs")
        Sr = {b: planes(f"S{b}r") for b in "so"}
        Si = {b: planes(f"S{b}i") for b in "so"}
        Ss = {b: planes(f"S{b}s") for b in "so"}

        wlhs = pp.tile([P, P], dt.float16, tag="wlhs", name="wlhs")
        wrhs = pp.tile([P, HN], dt.float16, tag="wrhs", name="wrhs")
        nc.vector.memset(wlhs[:], 0.0)
        nc.vector.memset(wrhs[:], 0.0)

        # ---- input DMAs in consumption order (P2s, P2o, P1s, P1o, P3s/P3o,
        # stage2-other) so matmul passes chase arrivals with no stalls ----
        # sync: tsi kt0-3, tsr kt0-3, vor kt0-3, to halves
        for t in range(NT):
            nc.sync.dma_start(TSiA[t], ts_d[:, HN * t:HN * (t + 1)])
        for t in range(NT):
            nc.sync.dma_start(TSrA[t], ts_d[:, B2 + HN * t:B2 + HN * (t + 1)])
        for t in range(NT):
            nc.sync.dma_start(VOrA[t], vo_d[:, B2 + HN * t:B2 + HN * (t + 1)])
        nc.sync.dma_start(TOa[:, 0:B2], to_d[:, 0:B2])
        nc.sync.dma_start(TOa[:, B2:4096], to_d[:, B2:4096])
        # gpsimd: vsi kt0-3, voi kt0-3, vsr kt0-3
        for t in range(NT):
            nc.gpsimd.dma_start(VSiA[t], vs_d[:, HN * t:HN * (t + 1)])
        for t in range(NT):
            nc.gpsimd.dma_start(VOiA[t], vo_d[:, HN * t:HN * (t + 1)])
        for t in range(NT):
            nc.gpsimd.dma_start(VSrA[t], vs_d[:, B2 + HN * t:B2 + HN * (t + 1)])

        _wn = [0]

        def warmup(n):
            for _ in range(n):
                i = _wn[0]
                _wn[0] += 1
                wp = pspool.tile([P, HN], dt.float32, tag=f"ps{6 + i % 2}", name=f"wps{i}")
                nc.tensor.matmul(wp[:], lhsT=wlhs[:], rhs=wrhs[:], start=True, stop=True)

        warmup(7)

        # sum planes: vector takes ts/vs (needed by P3s ~ mid-flight),
        # gpsimd takes vo/to (needed later)
        for t in range(NT):
            nc.vector.tensor_tensor(out=TSs[t][:], in0=TSrA[t], in1=TSiA[t], op=ALU.add)
            nc.vector.tensor_tensor(out=VSs[t][:], in0=VSrA[t], in1=VSiA[t], op=ALU.add)
        for t in range(NT):
            nc.gpsimd.tensor_tensor(out=VOs[t][:], in0=VOrA[t], in1=VOiA[t], op=ALU.add)
            nc.gpsimd.tensor_tensor(out=TOs[t][:], in0=TOrA[t][:],
                                    in1=TOiA[t][:], op=ALU.add)

        A = lambda ts_: [t[:] for t in ts_]
        TSsA, TOsA, VSsA, VOsA = A(TSs), A(TOs), A(VSs), A(VOs)
        SrA = {b: A(Sr[b]) for b in "so"}
        SiA = {b: A(Si[b]) for b in "so"}
        SsA = {b: A(Ss[b]) for b in "so"}

        # ---- stage 1, both b-blocks, stream-chasing kt-outer passes ----
        # banks: self-set mt -> bank mt, other-set mt -> bank 4+mt
        def acc_pass(ps, lhs, rhs):
            for kt in range(NT):
                st, sp = (kt == 0), (kt == NT - 1)
                for mt in range(NT):
                    msl = slice(P * mt, P * (mt + 1))
                    nc.tensor.matmul(ps[mt][:], lhsT=lhs[kt][:, msl], rhs=rhs[kt],
                                     start=st, stop=sp)

        p2S = [pspool.tile([P, HN], dt.float32, tag=f"ps{mt}", name=f"p2S{mt}")
               for mt in range(NT)]
        p2O = [pspool.tile([P, HN], dt.float32, tag=f"ps{4 + mt}", name=f"p2O{mt}")
               for mt in range(NT)]
        acc_pass(p2S, VSiA, TSiA)
        acc_pass(p2O, VOiA, TSiA)
        p2cS, p2cO = [], []
        for mt in range(NT):
            c = tp.tile([P, HN], dt.float32, tag="p2c", name=f"p2cS{mt}")
            nc.scalar.mul(c[:], p2S[mt][:], 1.0)
            p2cS.append(c)
        for mt in range(NT):
            c = tp.tile([P, HN], dt.float32, tag="p2co", name=f"p2cO{mt}")
            nc.scalar.mul(c[:], p2O[mt][:], 1.0)
            p2cO.append(c)
        p1S = [pspool.tile([P, HN], dt.float32, tag=f"ps{mt}", name=f"p1S{mt}")
               for mt in range(NT)]
        acc_pass(p1S, VSrA, TSrA)
        p1O = [pspool.tile([P, HN], dt.float32, tag=f"ps{4 + mt}", name=f"p1O{mt}")
               for mt in range(NT)]
        acc_pass(p1O, VOrA, TSrA)
        # Xr/t01 for self while P1o runs on the PE
        t01S, t01O = [], []
        for mt in range(NT):
            t01 = tp.tile([P, HN], dt.float32, tag="t01", name=f"t01S{mt}")
            nc.vector.tensor_tensor(out=SrA['s'][mt], in0=p1S[mt][:], in1=p2cS[mt][:],
                                    op=ALU.subtract)
            nc.vector.tensor_tensor(out=t01[:], in0=p1S[mt][:], in1=p2cS[mt][:], op=ALU.add)
            t01S.append(t01)
        # P3 self sweeps (bank mt), with per-mt combines trailing
        for mt in range(NT):
            msl = slice(P * mt, P * (mt + 1))
            p3 = pspool.tile([P, HN], dt.float32, tag=f"ps{mt}", name=f"p3S{mt}")
            for kt in range(NT):
                nc.tensor.matmul(p3[:], lhsT=VSsA[kt][:, msl], rhs=TSsA[kt],
                                 start=(kt == 0), stop=(kt == NT - 1))
            if mt == 0:
                for m2 in range(NT):
                    t01 = tp.tile([P, HN], dt.float32, tag="t01o", name=f"t01O{m2}")
                    nc.vector.tensor_tensor(out=SrA['o'][m2], in0=p1O[m2][:],
                                            in1=p2cO[m2][:], op=ALU.subtract)
                    nc.vector.tensor_tensor(out=t01[:], in0=p1O[m2][:], in1=p2cO[m2][:],
                                            op=ALU.add)
                    t01O.append(t01)
            nc.vector.tensor_tensor(out=SiA['s'][mt], in0=p3[:], in1=t01S[mt][:],
                                    op=ALU.subtract)
            nc.vector.tensor_tensor(out=SsA['s'][mt], in0=SrA['s'][mt], in1=SiA['s'][mt],
                                    op=ALU.add)
        for mt in range(NT):
            msl = slice(P * mt, P * (mt + 1))
            p3 = pspool.tile([P, HN], dt.float32, tag=f"ps{4 + mt}", name=f"p3O{mt}")
            for kt in range(NT):
                nc.tensor.matmul(p3[:], lhsT=VOsA[kt][:, msl], rhs=TSsA[kt],
                                 start=(kt == 0), stop=(kt == NT - 1))
            nc.vector.tensor_tensor(out=SiA['o'][mt], in0=p3[:], in1=t01O[mt][:],
                                    op=ALU.subtract)
            nc.vector.tensor_tensor(out=SsA['o'][mt], in0=SrA['o'][mt], in1=SiA['o'][mt],
                                    op=ALU.add)

        gctr = [0]

        def run_group(phase, mt, out_r, out_i, lhs_parts, rhs_parts,
                      split_tail=False, eager=None):
            g = gctr[0]
            gctr[0] += 1
            b0 = (3 * g) % 8
            lr, li, ls = lhs_parts
            rr, ri, rs = rhs_parts
            msl = slice(P * mt, P * (mt + 1))
            p2 = pspool.tile([P, HN], dt.float32, tag=f"ps{b0}", name=f"p2_{phase}_{mt}")
            p1 = pspool.tile([P, HN], dt.float32, tag=f"ps{(b0 + 1) % 8}", name=f"p1_{phase}_{mt}")
            for kt in range(NT):
                nc.tensor.matmul(p2[:], lhsT=li[kt][:, msl], rhs=ri[kt],
                                 start=(kt == 0), stop=(kt == NT - 1))
            p2c = tp.tile([P, HN], dt.float32, tag="p2c", name=f"p2c_{phase}_{mt}")
            if eager is None:
                nc.scalar.mul(p2c[:], p2[:], 1.0)
            else:
                # tail group: scalar-queue waits get coarsened to the final
                # matmul; vector's deps are precise, so stage psum there.
                nc.vector.tensor_copy(p2c[:], p2[:])
            for kt in range(NT):
                nc.tensor.matmul(p1[:], lhsT=lr[kt][:, msl], rhs=rr[kt],
                                 start=(kt == 0), stop=(kt == NT - 1))
            t01 = tp.tile([P, HN], dt.float32, tag="t01", name=f"t01_{phase}_{mt}")
            nc.vector.tensor_tensor(out=t01[:], in0=p1[:], in1=p2c[:], op=ALU.add)
            nc.vector.tensor_tensor(out=out_r, in0=p1[:], in1=p2c[:], op=ALU.subtract)
            if eager is not None:
                out_d, omsl, gtile = eager
                nc.sync.dma_start(out_d[omsl, 0:HN], gtile[:, 0:HN])
            halves = ((0, HN),) if not split_tail else ((0, HN // 2), (HN // 2, HN))
            for hi, (c0, c1) in enumerate(halves):
                wdt = c1 - c0
                full = (wdt == HN)
                p3 = pspool.tile([P, wdt], dt.float32,
                                 tag=f"ps{(b0 + 2) % 8}" if hi == 0 else f"ps{(b0 + 3) % 8}",
                                 name=f"p3_{phase}_{mt}_{hi}")
                for kt in range(NT):
                    nc.tensor.matmul(p3[:], lhsT=ls[kt][:, msl],
                                     rhs=rs[kt] if full else rs[kt][:, c0:c1],
                                     start=(kt == 0), stop=(kt == NT - 1))
                nc.vector.tensor_tensor(out=out_i if full else out_i[:, c0:c1],
                                        in0=p3[:],
                                        in1=t01[:] if full else t01[:, c0:c1],
                                        op=ALU.subtract)
                if eager is not None:
                    out_d, omsl, gtile = eager
                    nc.sync.dma_start(out_d[omsl, HN + c0:HN + c1],
                                      gtile[:, HN + c0:HN + c1])

        def stage2(phase, b, rhs_parts, out_d):
            for mt in range(NT):
                gtile = tp.tile([P, 1024], dt.float16, tag="gout", name=f"g_{phase}_{mt}")
                msl = slice(P * mt, P * (mt + 1))
                last = (phase == 3 and mt == NT - 1)
                run_group(phase, mt, gtile[:, 0:HN], gtile[:, HN:1024],
                          (SrA[b], SiA[b], SsA[b]), rhs_parts,
                          split_tail=(phase == 3 and mt >= NT - 2),
                          eager=(out_d, msl, gtile) if last else None)
                if not last:
                    nc.sync.dma_start(out_d[msl, :], gtile[:])

        stage2(2, 's', (TSrA, TSiA, TSsA), gs_d)
        stage2(3, 'o', (TOrA, TOiA, TOsA), go_d)

    nc.compile()
    return nc


def get_program():
    global _PROGRAM
    if _PROGRAM is None:
        _PROGRAM = build_program()
    return _PROGRAM


def kernel(field_real, field_imag, wavelengths):
    field_real = np.asarray(field_real)
    field_imag = np.asarray(field_imag)
    wavelengths = np.asarray(wavelengths)
    in_maps, meta = host_prepare(field_real, field_imag, wavelengths)
    from concourse.bass_utils import run_bass_kernel_spmd
    nc = get_program()
    res = run_bass_kernel_spmd(nc, in_maps, core_ids=list(range(8)))
    return assemble(res.results, meta)


if __name__ == "__main__":
    import jax
    import reference as ref
    cpu = jax.devices("cpu")[0]
    with jax.default_device(cpu):
        inputs = {k: np.asarray(v) for k, v in ref.setup_inputs().items()}
        expected = np.asarray(ref.reference(**{k: jax.device_put(v, cpu)
                                               for k, v in inputs.items()}))
    got = golden(np.asarray(inputs["field_real"]), np.asarray(inputs["field_imag"]),
                 np.asarray(inputs["wavelengths"]))
    err = np.abs(got - expected)
    print(f"golden absmax err {err.max():.4g} rel {err.max() / np.abs(expected).max():.4g}")


# revision 41
# speedup vs baseline: 1.1126x; 1.0002x over previous
"""Trainium2 Bass kernel for nn_CZT_prop: chirp-z (Bluestein) optical propagation.

Math: per wavelength both CZT axes share the transform M = diag(q) Tc diag(p)
with Tc[c,k] = tau(c-k), tau(d) = exp(-i*aw*d^2/2) an EVEN Toeplitz kernel, so
Tc is centrosymmetric and block-diagonalizes: Tc = K blockdiag(T+, T-) K / 2,
T+/-[c,k] = tau(c-k) +/- tau(c+k-1023), size 512.  The 2D result
    out = F0' . (Tc V Tc^T),   V = (field.F) * outer(p,p)
splits into four half-size quadrant products G_ab = T_a Vt_ab T_b (a,b in {+,-})
where Vt_ab are the +/- fold combos of V (host-prepared).  This HALVES the
device MACs vs the dense formulation.  The recombine (16 flip-adds), the F0'
multiply, and a rank-1 correction for the reference's zero-padded corner entry
[1023,0] of Tc are all host-side assembly.

Device per core (= one (wavelength, a-class)): two chained Karatsuba complex
matmul stages, contraction 512:
    S_b[j,c] = sum_k Vt_ab[k,j] T_a[k,c]     (b in {self, other})
    G_ab[c,d] = sum_j S_b[j,c] T_b[j,d]
192 fp16 matmuls of [128x512]@K=128 = 98304 PE cycles.  Sharding: 8 cores =
4 wavelengths x 2 centro-classes.  Zero communication.
"""
import math
import numpy as np

f32 = np.float32
f16 = np.float16
f64 = np.float64

# ---- static geometry (mirrors the problem spec) ----
H = 1024
M = 1024
N_WL = 4
DX = 100e-6
ODX = 10e-6
ODY = 10e-6
Z = 0.05
TWO_PI = 2.0 * np.pi
P = 128
HN = 512
NT = 4          # 128-row tiles per 512
X_IN = np.linspace(-H * DX / 2, H * DX / 2, H).astype(f64)
X_OUT = np.linspace(-M * ODX / 2, M * ODX / 2, M).astype(f64)


def _pow2_below(x):
    return 2.0 ** math.floor(math.log2(x))


def czt_factors(wl):
    """q[c], p[k], aw for the 1-axis CZT: out[c] = q[c] sum_k tau(c-k) p[k] x[k]."""
    Dm = wl * Z / DX
    f1 = X_OUT[0] + Dm / 2
    f2 = X_OUT[-1] + Dm / 2
    D1 = f1 + (M * Dm + f2 - f1) / (2 * M)
    D2 = f2 + (M * Dm + f2 - f1) / (2 * M)
    alpha_A = TWO_PI * D1 / Dm
    aw = -TWO_PI * (D1 - D2) / (M * Dm)
    k = np.arange(H, dtype=f64)
    c = np.arange(M, dtype=f64)
    h = lambda x: np.exp(1j * aw * x * x / 2)
    l = c / M * (D2 - D1) + D1
    m_shift = np.exp(-1j * TWO_PI * l * (-H / 2 + 0.5) / Dm)
    q = m_shift * h(c) * np.exp(-1j * aw * c) * np.exp(-1j * aw / 2)
    p = np.exp(-1j * alpha_A * k) * h(k) * np.exp(1j * aw * k)
    return q, p, aw


def _rs_kernel_full(xg, wl):
    """RS transfer kernel on the full plane via its 4-fold quad symmetry."""
    kv = TWO_PI / wl
    xh = xg[:HN]
    r2 = xh[:, None] ** 2 + xh[None, :] ** 2 + Z * Z
    r = np.sqrt(r2)
    aa = (Z / TWO_PI) / (r2 * r)
    bt = (kv * Z / TWO_PI) / r2
    ph = kv * r
    cq, sq = np.cos(ph), np.sin(ph)
    Fq = (aa * cq + bt * sq) + 1j * (aa * sq - bt * cq)
    return np.block([[Fq, Fq[:, ::-1]], [Fq[::-1, :], Fq[::-1, ::-1]]])


def host_prepare(field_real, field_imag, wavelengths):
    """Per-core device inputs + assembly metadata.  All f64 host math."""
    wls = np.asarray(wavelengths, f64)
    Jr = np.arange(HN)[::-1]
    in_maps = [None] * 8
    meta = []
    for w in range(N_WL):
        wl = f64(wls[w])
        q, p, aw = czt_factors(wl)
        tau = lambda d: np.exp(-1j * aw * np.asarray(d, f64) ** 2 / 2)
        F = _rs_kernel_full(X_IN, wl)
        F0 = _rs_kernel_full(X_OUT, wl)
        U = (np.asarray(field_real[0, w], f64)
             + 1j * np.asarray(field_imag[0, w], f64)) * F
        V = U * np.outer(p, p)

        cc = np.arange(HN, dtype=f64)[:, None]
        kk = np.arange(HN, dtype=f64)[None, :]
        tnear = tau(cc - kk)
        tfar = tau(cc + kk - (H - 1))
        Tp = tnear + tfar
        Tm = tnear - tfar

        V11 = V[:HN, :HN]; V12 = V[:HN, HN:]
        V21 = V[HN:, :HN]; V22 = V[HN:, HN:]
        A12 = V12[:, Jr]; A21 = V21[Jr, :]; A22 = V22[Jr][:, Jr]
        Vt = {('+', '+'): (V11 + A12 + A21 + A22) / 4,
              ('+', '-'): (V11 - A12 + A21 - A22) / 4,
              ('-', '+'): (V11 + A12 - A21 - A22) / 4,
              ('-', '-'): (V11 - A12 - A21 + A22) / 4}

        # pow2 scaling: one scale per (a,b) block, chosen so BOTH stage
        # outputs stay in fp16 range with ~8x headroom (stage-2 dominates).
        mt2 = {'+': float(np.mean(np.abs(Tp) ** 2)),
               '-': float(np.mean(np.abs(Tm) ** 2))}
        scales = {}
        for ab, Vab in Vt.items():
            fro2 = float(np.sum(np.abs(Vab) ** 2))
            s2_raw = 8.0 * math.sqrt(0.5 * mt2[ab[0]] * mt2[ab[1]] * fro2)
            scales[ab] = _pow2_below(8192.0 / max(s2_raw, 1e-300))

        def detile(z):
            """complex [512,512] -> [128, 4096]: i-plane kt-blocks then r."""
            zi = f16(z.imag); zr = f16(z.real)
            blocks = [zi[P * t:P * (t + 1), :] for t in range(NT)] \
                   + [zr[P * t:P * (t + 1), :] for t in range(NT)]
            return np.ascontiguousarray(np.concatenate(blocks, axis=1))

        Tdev = {'+': detile(Tp), '-': detile(Tm)}
        for ai, a in enumerate(('+', '-')):
            b_self, b_oth = a, ('-' if a == '+' else '+')
            vs = Vt[(a, b_self)] * scales[(a, b_self)]
            vo = Vt[(a, b_oth)] * scales[(a, b_oth)]
            in_maps[2 * w + ai] = {
                "ts": Tdev[a],
                "to": Tdev[b_oth],
                "vs": detile(vs),
                "vo": detile(vo),
            }

        # rank-1 corner correction (f64, exact): out = Tc_ref V Tc_ref^T,
        # Tc_ref = Tc - s e_1023 e_0^T.  Row/col vectors via T+/- blocks:
        # T11 = (Tp+Tm)/2, H = (Tp-Tm)/2 (T12 = H J, T21 = J H, T22 = J T11 J)
        s = tau(f64(H - 1))
        T11 = (Tp + Tm) / 2
        Hh = (Tp - Tm) / 2
        v1 = V[0, :HN]; v2r = V[0, HN:][Jr]
        row = np.empty(M, np.complex128)
        row[:HN] = v1 @ T11 + v2r @ Hh
        row[HN:] = (v1 @ Hh + v2r @ T11)[Jr]
        u1 = V[:HN, 0]; u2r = V[HN:, 0][Jr]
        col = np.empty(M, np.complex128)
        col[:HN] = T11 @ u1 + Hh @ u2r
        col[HN:] = (Hh @ u1 + T11 @ u2r)[Jr]
        # F0' with all diagonal factors + global scale folded
        F0p = F0 * np.outer(q, q) * (Z * ODX * ODY * wl)
        meta.append({
            "w": w,
            "scales": scales,
            "F0p": F0p.astype(np.complex128),
            "corr_row": s * row, "corr_col": s * col,
            "corr_s": s * s * V[0, 0],
        })
    return in_maps, meta


def assemble(results, meta):
    out = np.zeros((1, N_WL, M, M), np.complex64)
    Jr = np.arange(HN)[::-1]
    for md in meta:
        w = md["w"]
        qs = {}
        for ai, a in enumerate(('+', '-')):
            r = results[2 * w + ai]
            b_self, b_oth = a, ('-' if a == '+' else '+')
            for key, ab in (("gs", (a, b_self)), ("go", (a, b_oth))):
                g = r[key].astype(f32)
                qs[ab] = (g[:, :HN].astype(f64) + 1j * g[:, HN:].astype(f64)) \
                    / md["scales"][ab]
        Q1 = qs[('+', '+')]; Q2 = qs[('+', '-')]
        Q3 = qs[('-', '+')]; Q4 = qs[('-', '-')]
        Gf = np.empty((M, M), np.complex128)
        Gf[:HN, :HN] = Q1 + Q2 + Q3 + Q4
        Gf[:HN, HN:] = (Q1 - Q2 + Q3 - Q4)[:, Jr]
        Gf[HN:, :HN] = (Q1 + Q2 - Q3 - Q4)[Jr, :]
        Gf[HN:, HN:] = (Q1 - Q2 - Q3 + Q4)[Jr][:, Jr]
        Gf[M - 1, :] -= md["corr_row"]
        Gf[:, M - 1] -= md["corr_col"]
        Gf[M - 1, M - 1] += md["corr_s"]
        out[0, w] = (md["F0p"] * Gf).astype(np.complex64)
    return out


# ---------------- golden (numpy) model of the device program ----------------

def golden_core(inp):
    def split(x):
        zi = np.vstack([x[:, HN * t:HN * (t + 1)] for t in range(NT)])
        zr = np.vstack([x[:, HN * (NT + t):HN * (NT + t + 1)] for t in range(NT)])
        return zr, zi

    def karatsuba(Ar, Ai, As, Br, Bi, Bs):
        P1 = Ar.astype(f32).T @ Br.astype(f32)
        P2 = Ai.astype(f32).T @ Bi.astype(f32)
        P3 = As.astype(f32).T @ Bs.astype(f32)
        Xr = f16(P1 - P2)
        Xi = f16(P3 - f32(P1 + P2))
        return Xr, Xi

    tsr, tsi = split(inp["ts"]); tss = f16(tsr.astype(f32) + tsi.astype(f32))
    tor_, toi = split(inp["to"]); tos = f16(tor_.astype(f32) + toi.astype(f32))
    out = {}
    for key, vkey, (br, bi, bs) in (("gs", "vs", (tsr, tsi, tss)),
                                    ("go", "vo", (tor_, toi, tos))):
        vr, vi = split(inp[vkey]); vv = f16(vr.astype(f32) + vi.astype(f32))
        Sr, Si = karatsuba(vr, vi, vv, tsr, tsi, tss)
        Ss = f16(Sr.astype(f32) + Si.astype(f32))
        Gr, Gi = karatsuba(Sr, Si, Ss, br, bi, bs)
        out[key] = np.concatenate([Gr, Gi], axis=1)
    return out


def golden(field_real, field_imag, wavelengths):
    in_maps, meta = host_prepare(field_real, field_imag, wavelengths)
    results = [golden_core(m) for m in in_maps]
    return assemble(results, meta)


# ---------------- bass program ----------------

_PROGRAM = None


def build_program():
    import concourse.bass as bass
    import concourse.tile as tile
    import concourse.mybir as mybir
    from concourse import bacc

    dt = mybir.dt
    ALU = mybir.AluOpType

    nc = bacc.Bacc("TRN2", target_bir_lowering=False, debug=False, num_devices=8)

    ts_d = nc.dram_tensor("ts", [P, 4096], dt.float16, kind="ExternalInput").ap()
    to_d = nc.dram_tensor("to", [P, 4096], dt.float16, kind="ExternalInput").ap()
    vs_d = nc.dram_tensor("vs", [P, 4096], dt.float16, kind="ExternalInput").ap()
    vo_d = nc.dram_tensor("vo", [P, 4096], dt.float16, kind="ExternalInput").ap()
    gs_d = nc.dram_tensor("gs", [HN, 1024], dt.float16, kind="ExternalOutput").ap()
    go_d = nc.dram_tensor("go", [HN, 1024], dt.float16, kind="ExternalOutput").ap()

    B2 = NT * HN  # 2048

    with tile.TileContext(nc) as tc:
      with tc.tile_pool(name="persist", bufs=1) as pp, \
           tc.tile_pool(name="psum", bufs=1, space="PSUM") as pspool, \
           tc.tile_pool(name="tmp", bufs=4) as tp:

        def bigtile(nm):
            return pp.tile([P, 4096], dt.float16, tag=nm, name=nm)
        TSa, TOa, VSa, VOa = bigtile("TSa"), bigtile("TOa"), bigtile("VSa"), bigtile("VOa")

        def plane_aps(big):
            iA = [big[:, HN * t:HN * (t + 1)] for t in range(NT)]
            rA = [big[:, B2 + HN * t:B2 + HN * (t + 1)] for t in range(NT)]
            return rA, iA
        TSrA, TSiA = plane_aps(TSa)
        TOrA, TOiA = plane_aps(TOa)
        VSrA, VSiA = plane_aps(VSa)
        VOrA, VOiA = plane_aps(VOa)

        def planes(nm):
            return [pp.tile([P, HN], dt.float16, tag=f"{nm}{t}", name=f"{nm}{t}")
                    for t in range(NT)]
        TSs, TOs, VSs, VOs = planes("TSs"), planes("TOs"), planes("VSs"), planes("VOs")
        Sr = {b: planes(f"S{b}r") for b in "so"}
        Si = {b: planes(f"S{b}i") for b in "so"}
        Ss = {b: planes(f"S{b}s") for b in "so"}

        wlhs = pp.tile([P, P], dt.float16, tag="wlhs", name="wlhs")
        wrhs = pp.tile([P, HN], dt.float16, tag="wrhs", name="wrhs")
        nc.vector.memset(wlhs[:], 0.0)
        nc.vector.memset(wrhs[:], 0.0)

        # ---- input DMAs in consumption order (P2s, P2o, P1s, P1o, P3s/P3o,
        # stage2-other) so matmul passes chase arrivals with no stalls ----
        # sync: tsi kt0-3, tsr kt0-3, vor kt0-3, to halves
        for t in range(NT):
            nc.sync.dma_start(TSiA[t], ts_d[:, HN * t:HN * (t + 1)])
        for t in range(NT):
            nc.sync.dma_start(TSrA[t], ts_d[:, B2 + HN * t:B2 + HN * (t + 1)])
        for t in range(NT):
            nc.sync.dma_start(VOrA[t], vo_d[:, B2 + HN * t:B2 + HN * (t + 1)])
        nc.sync.dma_start(TOa[:, 0:B2], to_d[:, 0:B2])
        nc.sync.dma_start(TOa[:, B2:4096], to_d[:, B2:4096])
        # gpsimd: vsi kt0-3, voi kt0-3, vsr kt0-3
        for t in range(NT):
            nc.gpsimd.dma_start(VSiA[t], vs_d[:, HN * t:HN * (t + 1)])
        for t in range(NT):
            nc.gpsimd.dma_start(VOiA[t], vo_d[:, HN * t:HN * (t + 1)])
        for t in range(NT):
            nc.gpsimd.dma_start(VSrA[t], vs_d[:, B2 + HN * t:B2 + HN * (t + 1)])

        _wn = [0]

        def warmup(n):
            for _ in range(n):
                i = _wn[0]
                _wn[0] += 1
                wp = pspool.tile([P, HN], dt.float32, tag=f"ps{6 + i % 2}", name=f"wps{i}")
                nc.tensor.matmul(wp[:], lhsT=wlhs[:], rhs=wrhs[:], start=True, stop=True)

        warmup(8)

        # sum planes: vector takes ts/vs (needed by P3s ~ mid-flight),
        # gpsimd takes vo/to (needed later)
        for t in range(NT):
            nc.vector.tensor_tensor(out=TSs[t][:], in0=TSrA[t], in1=TSiA[t], op=ALU.add)
            nc.vector.tensor_tensor(out=VSs[t][:], in0=VSrA[t], in1=VSiA[t], op=ALU.add)
        for t in range(NT):
            nc.gpsimd.tensor_tensor(out=VOs[t][:], in0=VOrA[t], in1=VOiA[t], op=ALU.add)
            nc.gpsimd.tensor_tensor(out=TOs[t][:], in0=TOrA[t][:],
                                    in1=TOiA[t][:], op=ALU.add)

        A = lambda ts_: [t[:] for t in ts_]
        TSsA, TOsA, VSsA, VOsA = A(TSs), A(TOs), A(VSs), A(VOs)
        SrA = {b: A(Sr[b]) for b in "so"}
        SiA = {b: A(Si[b]) for b in "so"}
        SsA = {b: A(Ss[b]) for b in "so"}

        # ---- stage 1, both b-blocks, stream-chasing kt-outer passes ----
        # banks: self-set mt -> bank mt, other-set mt -> bank 4+mt
        def acc_pass(ps, lhs, rhs):
            for kt in range(NT):
                st, sp = (kt == 0), (kt == NT - 1)
                for mt in range(NT):
                    msl = slice(P * mt, P * (mt + 1))
                    nc.tensor.matmul(ps[mt][:], lhsT=lhs[kt][:, msl], rhs=rhs[kt],
                                     start=st, stop=sp)

        p2S = [pspool.tile([P, HN], dt.float32, tag=f"ps{mt}", name=f"p2S{mt}")
               for mt in range(NT)]
        p2O = [pspool.tile([P, HN], dt.float32, tag=f"ps{4 + mt}", name=f"p2O{mt}")
               for mt in range(NT)]
        acc_pass(p2S, VSiA, TSiA)
        acc_pass(p2O, VOiA, TSiA)
        p2cS, p2cO = [], []
        for mt in range(NT):
            c = tp.tile([P, HN], dt.float32, tag="p2c", name=f"p2cS{mt}")
            nc.scalar.mul(c[:], p2S[mt][:], 1.0)
            p2cS.append(c)
        for mt in range(NT):
            c = tp.tile([P, HN], dt.float32, tag="p2co", name=f"p2cO{mt}")
            nc.scalar.mul(c[:], p2O[mt][:], 1.0)
            p2cO.append(c)
        p1S = [pspool.tile([P, HN], dt.float32, tag=f"ps{mt}", name=f"p1S{mt}")
               for mt in range(NT)]
        acc_pass(p1S, VSrA, TSrA)
        p1O = [pspool.tile([P, HN], dt.float32, tag=f"ps{4 + mt}", name=f"p1O{mt}")
               for mt in range(NT)]
        acc_pass(p1O, VOrA, TSrA)
        # Xr/t01 for self while P1o runs on the PE
        t01S, t01O = [], []
        for mt in range(NT):
            t01 = tp.tile([P, HN], dt.float32, tag="t01", name=f"t01S{mt}")
            nc.vector.tensor_tensor(out=SrA['s'][mt], in0=p1S[mt][:], in1=p2cS[mt][:],
                                    op=ALU.subtract)
            nc.vector.tensor_tensor(out=t01[:], in0=p1S[mt][:], in1=p2cS[mt][:], op=ALU.add)
            t01S.append(t01)
        # P3 self sweeps (bank mt), with per-mt combines trailing
        for mt in range(NT):
            msl = slice(P * mt, P * (mt + 1))
            p3 = pspool.tile([P, HN], dt.float32, tag=f"ps{mt}", name=f"p3S{mt}")
            for kt in range(NT):
                nc.tensor.matmul(p3[:], lhsT=VSsA[kt][:, msl], rhs=TSsA[kt],
                                 start=(kt == 0), stop=(kt == NT - 1))
            if mt == 0:
                for m2 in range(NT):
                    t01 = tp.tile([P, HN], dt.float32, tag="t01o", name=f"t01O{m2}")
                    nc.vector.tensor_tensor(out=SrA['o'][m2], in0=p1O[m2][:],
                                            in1=p2cO[m2][:], op=ALU.subtract)
                    nc.vector.tensor_tensor(out=t01[:], in0=p1O[m2][:], in1=p2cO[m2][:],
                                            op=ALU.add)
                    t01O.append(t01)
            nc.vector.tensor_tensor(out=SiA['s'][mt], in0=p3[:], in1=t01S[mt][:],
                                    op=ALU.subtract)
            nc.vector.tensor_tensor(out=SsA['s'][mt], in0=SrA['s'][mt], in1=SiA['s'][mt],
                                    op=ALU.add)
        for mt in range(NT):
            msl = slice(P * mt, P * (mt + 1))
            p3 = pspool.tile([P, HN], dt.float32, tag=f"ps{4 + mt}", name=f"p3O{mt}")
            for kt in range(NT):
                nc.tensor.matmul(p3[:], lhsT=VOsA[kt][:, msl], rhs=TSsA[kt],
                                 start=(kt == 0), stop=(kt == NT - 1))
            nc.vector.tensor_tensor(out=SiA['o'][mt], in0=p3[:], in1=t01O[mt][:],
                                    op=ALU.subtract)
            nc.vector.tensor_tensor(out=SsA['o'][mt], in0=SrA['o'][mt], in1=SiA['o'][mt],
                                    op=ALU.add)

        gctr = [0]

        def run_group(phase, mt, out_r, out_i, lhs_parts, rhs_parts,
                      split_tail=False, eager=None):
            g = gctr[0]
            gctr[0] += 1
            b0 = (3 * g) % 8
            lr, li, ls = lhs_parts
            rr, ri, rs = rhs_parts
            msl = slice(P * mt, P * (mt + 1))
            p2 = pspool.tile([P, HN], dt.float32, tag=f"ps{b0}", name=f"p2_{phase}_{mt}")
            p1 = pspool.tile([P, HN], dt.float32, tag=f"ps{(b0 + 1) % 8}", name=f"p1_{phase}_{mt}")
            for kt in range(NT):
                nc.tensor.matmul(p2[:], lhsT=li[kt][:, msl], rhs=ri[kt],
                                 start=(kt == 0), stop=(kt == NT - 1))
            p2c = tp.tile([P, HN], dt.float32, tag="p2c", name=f"p2c_{phase}_{mt}")
            if eager is None:
                nc.scalar.mul(p2c[:], p2[:], 1.0)
            else:
                # tail group: scalar-queue waits get coarsened to the final
                # matmul; vector's deps are precise, so stage psum there.
                nc.vector.tensor_copy(p2c[:], p2[:])
            for kt in range(NT):
                nc.tensor.matmul(p1[:], lhsT=lr[kt][:, msl], rhs=rr[kt],
                                 start=(kt == 0), stop=(kt == NT - 1))
            t01 = tp.tile([P, HN], dt.float32, tag="t01", name=f"t01_{phase}_{mt}")
            nc.vector.tensor_tensor(out=t01[:], in0=p1[:], in1=p2c[:], op=ALU.add)
            nc.vector.tensor_tensor(out=out_r, in0=p1[:], in1=p2c[:], op=ALU.subtract)
            if eager is not None:
                out_d, omsl, gtile = eager
                nc.sync.dma_start(out_d[omsl, 0:HN], gtile[:, 0:HN])
            halves = ((0, HN),) if not split_tail else ((0, HN // 2), (HN // 2, HN))
            for hi, (c0, c1) in enumerate(halves):
                wdt = c1 - c0
                full = (wdt == HN)
                p3 = pspool.tile([P, wdt], dt.float32,
                                 tag=f"ps{(b0 + 2) % 8}" if hi == 0 else f"ps{(b0 + 3) % 8}",
                                 name=f"p3_{phase}_{mt}_{hi}")
                for kt in range(NT):
                    nc.tensor.matmul(p3[:], lhsT=ls[kt][:, msl],
                                     rhs=rs[kt] if full else rs[kt][:, c0:c1],
                                     start=(kt == 0), stop=(kt == NT - 1))
                nc.vector.tensor_tensor(out=out_i if full else out_i[:, c0:c1],
                                        in0=p3[:],
                                        in1=t01[:] if full else t01[:, c0:c1],
                                        op=ALU.subtract)
                if eager is not None:
                    out_d, omsl, gtile = eager
                    nc.sync.dma_start(out_d[omsl, HN + c0:HN + c1],
                                      gtile[:, HN + c0:HN + c1])

        def stage2(phase, b, rhs_parts, out_d, mts):
            for mt in mts:
                gtile = tp.tile([P, 1024], dt.float16, tag="gout", name=f"g_{phase}_{mt}")
                msl = slice(P * mt, P * (mt + 1))
                run_group(phase, mt, gtile[:, 0:HN], gtile[:, HN:1024],
                          (SrA[b], SiA[b], SsA[b]), rhs_parts)
                nc.sync.dma_start(out_d[msl, :], gtile[:])

        stage2(2, 's', (TSrA, TSiA, TSsA), gs_d, range(NT))
        stage2(3, 'o', (TOrA, TOiA, TOsA), go_d, (0, 1))

        # ---- last two groups: P2/P1 sweeps front-loaded with hand-picked
        # early-free banks so the tail is only P3 halves + Gi + DMA ----
        BK = {2: (2, 3, 4, 5), 3: (6, 7, 0, 1)}
        lp2, lp2c, lp1, lt01, lgt = {}, {}, {}, {}, {}
        for mt in (2, 3):
            msl = slice(P * mt, P * (mt + 1))
            lp2[mt] = pspool.tile([P, HN], dt.float32, tag=f"ps{BK[mt][0]}",
                                  name=f"lp2_{mt}")
            for kt in range(NT):
                nc.tensor.matmul(lp2[mt][:], lhsT=SiA['o'][kt][:, msl], rhs=TOiA[kt],
                                 start=(kt == 0), stop=(kt == NT - 1))
            c = tp.tile([P, HN], dt.float32, tag="p2c", name=f"lp2c_{mt}")
            if mt == 2:
                nc.scalar.mul(c[:], lp2[mt][:], 1.0)
            else:
                nc.vector.tensor_copy(c[:], lp2[mt][:])
            lp2c[mt] = c
        for mt in (2, 3):
            msl = slice(P * mt, P * (mt + 1))
            lp1[mt] = pspool.tile([P, HN], dt.float32, tag=f"ps{BK[mt][1]}",
                                  name=f"lp1_{mt}")
            for kt in range(NT):
                nc.tensor.matmul(lp1[mt][:], lhsT=SrA['o'][kt][:, msl], rhs=TOrA[kt],
                                 start=(kt == 0), stop=(kt == NT - 1))
        for mt in (2, 3):
            msl = slice(P * mt, P * (mt + 1))
            gtile = tp.tile([P, 1024], dt.float16, tag="gout", name=f"lg_{mt}")
            t01 = tp.tile([P, HN], dt.float32, tag="t01", name=f"lt01_{mt}")
            nc.vector.tensor_tensor(out=t01[:], in0=lp1[mt][:], in1=lp2c[mt][:],
                                    op=ALU.add)
            nc.vector.tensor_tensor(out=gtile[:, 0:HN], in0=lp1[mt][:],
                                    in1=lp2c[mt][:], op=ALU.subtract)
            nc.sync.dma_start(go_d[msl, 0:HN], gtile[:, 0:HN])
            lt01[mt], lgt[mt] = t01, gtile
        for mt in (2, 3):
            msl = slice(P * mt, P * (mt + 1))
            for hi, (c0, c1) in enumerate(((0, HN // 2), (HN // 2, HN))):
                p3 = pspool.tile([P, HN // 2], dt.float32, tag=f"ps{BK[mt][2 + hi]}",
                                 name=f"lp3_{mt}_{hi}")
                for kt in range(NT):
                    nc.tensor.matmul(p3[:], lhsT=SsA['o'][kt][:, msl],
                                     rhs=TOsA[kt][:, c0:c1],
                                     start=(kt == 0), stop=(kt == NT - 1))
                nc.vector.tensor_tensor(out=lgt[mt][:, HN + c0:HN + c1], in0=p3[:],
                                        in1=lt01[mt][:, c0:c1], op=ALU.subtract)
                nc.sync.dma_start(go_d[msl, HN + c0:HN + c1],
                                  lgt[mt][:, HN + c0:HN + c1])

    nc.compile()
    return nc


def get_program():
    global _PROGRAM
    if _PROGRAM is None:
        _PROGRAM = build_program()
    return _PROGRAM


def kernel(field_real, field_imag, wavelengths):
    field_real = np.asarray(field_real)
    field_imag = np.asarray(field_imag)
    wavelengths = np.asarray(wavelengths)
    in_maps, meta = host_prepare(field_real, field_imag, wavelengths)
    from concourse.bass_utils import run_bass_kernel_spmd
    nc = get_program()
    res = run_bass_kernel_spmd(nc, in_maps, core_ids=list(range(8)))
    return assemble(res.results, meta)


if __name__ == "__main__":
    import jax
    import reference as ref
    cpu = jax.devices("cpu")[0]
    with jax.default_device(cpu):
        inputs = {k: np.asarray(v) for k, v in ref.setup_inputs().items()}
        expected = np.asarray(ref.reference(**{k: jax.device_put(v, cpu)
                                               for k, v in inputs.items()}))
    got = golden(np.asarray(inputs["field_real"]), np.asarray(inputs["field_imag"]),
                 np.asarray(inputs["wavelengths"]))
    err = np.abs(got - expected)
    print(f"golden absmax err {err.max():.4g} rel {err.max() / np.abs(expected).max():.4g}")


# revision 42
# speedup vs baseline: 1.1254x; 1.0115x over previous
"""Trainium2 Bass kernel for nn_CZT_prop: chirp-z (Bluestein) optical propagation.

Math: per wavelength both CZT axes share the transform M = diag(q) Tc diag(p)
with Tc[c,k] = tau(c-k), tau(d) = exp(-i*aw*d^2/2) an EVEN Toeplitz kernel, so
Tc is centrosymmetric and block-diagonalizes: Tc = K blockdiag(T+, T-) K / 2,
T+/-[c,k] = tau(c-k) +/- tau(c+k-1023), size 512.  The 2D result
    out = F0' . (Tc V Tc^T),   V = (field.F) * outer(p,p)
splits into four half-size quadrant products G_ab = T_a Vt_ab T_b (a,b in {+,-})
where Vt_ab are the +/- fold combos of V (host-prepared).  This HALVES the
device MACs vs the dense formulation.  The recombine (16 flip-adds), the F0'
multiply, and a rank-1 correction for the reference's zero-padded corner entry
[1023,0] of Tc are all host-side assembly.

Device per core (= one (wavelength, a-class)): two chained Karatsuba complex
matmul stages, contraction 512:
    S_b[j,c] = sum_k Vt_ab[k,j] T_a[k,c]     (b in {self, other})
    G_ab[c,d] = sum_j S_b[j,c] T_b[j,d]
192 fp16 matmuls of [128x512]@K=128 = 98304 PE cycles.  Sharding: 8 cores =
4 wavelengths x 2 centro-classes.  Zero communication.
"""
import math
import numpy as np

f32 = np.float32
f16 = np.float16
f64 = np.float64

# ---- static geometry (mirrors the problem spec) ----
H = 1024
M = 1024
N_WL = 4
DX = 100e-6
ODX = 10e-6
ODY = 10e-6
Z = 0.05
TWO_PI = 2.0 * np.pi
P = 128
HN = 512
NT = 4          # 128-row tiles per 512
X_IN = np.linspace(-H * DX / 2, H * DX / 2, H).astype(f64)
X_OUT = np.linspace(-M * ODX / 2, M * ODX / 2, M).astype(f64)


def _pow2_below(x):
    return 2.0 ** math.floor(math.log2(x))


def czt_factors(wl):
    """q[c], p[k], aw for the 1-axis CZT: out[c] = q[c] sum_k tau(c-k) p[k] x[k]."""
    Dm = wl * Z / DX
    f1 = X_OUT[0] + Dm / 2
    f2 = X_OUT[-1] + Dm / 2
    D1 = f1 + (M * Dm + f2 - f1) / (2 * M)
    D2 = f2 + (M * Dm + f2 - f1) / (2 * M)
    alpha_A = TWO_PI * D1 / Dm
    aw = -TWO_PI * (D1 - D2) / (M * Dm)
    k = np.arange(H, dtype=f64)
    c = np.arange(M, dtype=f64)
    h = lambda x: np.exp(1j * aw * x * x / 2)
    l = c / M * (D2 - D1) + D1
    m_shift = np.exp(-1j * TWO_PI * l * (-H / 2 + 0.5) / Dm)
    q = m_shift * h(c) * np.exp(-1j * aw * c) * np.exp(-1j * aw / 2)
    p = np.exp(-1j * alpha_A * k) * h(k) * np.exp(1j * aw * k)
    return q, p, aw


def _rs_kernel_full(xg, wl):
    """RS transfer kernel on the full plane via its 4-fold quad symmetry."""
    kv = TWO_PI / wl
    xh = xg[:HN]
    r2 = xh[:, None] ** 2 + xh[None, :] ** 2 + Z * Z
    r = np.sqrt(r2)
    aa = (Z / TWO_PI) / (r2 * r)
    bt = (kv * Z / TWO_PI) / r2
    ph = kv * r
    cq, sq = np.cos(ph), np.sin(ph)
    Fq = (aa * cq + bt * sq) + 1j * (aa * sq - bt * cq)
    return np.block([[Fq, Fq[:, ::-1]], [Fq[::-1, :], Fq[::-1, ::-1]]])


def host_prepare(field_real, field_imag, wavelengths):
    """Per-core device inputs + assembly metadata.  All f64 host math."""
    wls = np.asarray(wavelengths, f64)
    Jr = np.arange(HN)[::-1]
    in_maps = [None] * 8
    meta = []
    for w in range(N_WL):
        wl = f64(wls[w])
        q, p, aw = czt_factors(wl)
        tau = lambda d: np.exp(-1j * aw * np.asarray(d, f64) ** 2 / 2)
        F = _rs_kernel_full(X_IN, wl)
        F0 = _rs_kernel_full(X_OUT, wl)
        U = (np.asarray(field_real[0, w], f64)
             + 1j * np.asarray(field_imag[0, w], f64)) * F
        V = U * np.outer(p, p)

        cc = np.arange(HN, dtype=f64)[:, None]
        kk = np.arange(HN, dtype=f64)[None, :]
        tnear = tau(cc - kk)
        tfar = tau(cc + kk - (H - 1))
        Tp = tnear + tfar
        Tm = tnear - tfar

        V11 = V[:HN, :HN]; V12 = V[:HN, HN:]
        V21 = V[HN:, :HN]; V22 = V[HN:, HN:]
        A12 = V12[:, Jr]; A21 = V21[Jr, :]; A22 = V22[Jr][:, Jr]
        Vt = {('+', '+'): (V11 + A12 + A21 + A22) / 4,
              ('+', '-'): (V11 - A12 + A21 - A22) / 4,
              ('-', '+'): (V11 + A12 - A21 - A22) / 4,
              ('-', '-'): (V11 - A12 - A21 + A22) / 4}

        # pow2 scaling: one scale per (a,b) block, chosen so BOTH stage
        # outputs stay in fp16 range with ~8x headroom (stage-2 dominates).
        mt2 = {'+': float(np.mean(np.abs(Tp) ** 2)),
               '-': float(np.mean(np.abs(Tm) ** 2))}
        scales = {}
        for ab, Vab in Vt.items():
            fro2 = float(np.sum(np.abs(Vab) ** 2))
            s2_raw = 8.0 * math.sqrt(0.5 * mt2[ab[0]] * mt2[ab[1]] * fro2)
            scales[ab] = _pow2_below(8192.0 / max(s2_raw, 1e-300))

        def detile(z):
            """complex [512,512] -> [128, 4096]: i-plane kt-blocks then r."""
            zi = f16(z.imag); zr = f16(z.real)
            blocks = [zi[P * t:P * (t + 1), :] for t in range(NT)] \
                   + [zr[P * t:P * (t + 1), :] for t in range(NT)]
            return np.ascontiguousarray(np.concatenate(blocks, axis=1))

        Tdev = {'+': detile(Tp), '-': detile(Tm)}
        for ai, a in enumerate(('+', '-')):
            b_self, b_oth = a, ('-' if a == '+' else '+')
            vs = Vt[(a, b_self)] * scales[(a, b_self)]
            vo = Vt[(a, b_oth)] * scales[(a, b_oth)]
            in_maps[2 * w + ai] = {
                "ts": Tdev[a],
                "to": Tdev[b_oth],
                "vs": detile(vs),
                "vo": detile(vo),
            }

        # rank-1 corner correction (f64, exact): out = Tc_ref V Tc_ref^T,
        # Tc_ref = Tc - s e_1023 e_0^T.  Row/col vectors via T+/- blocks:
        # T11 = (Tp+Tm)/2, H = (Tp-Tm)/2 (T12 = H J, T21 = J H, T22 = J T11 J)
        s = tau(f64(H - 1))
        T11 = (Tp + Tm) / 2
        Hh = (Tp - Tm) / 2
        v1 = V[0, :HN]; v2r = V[0, HN:][Jr]
        row = np.empty(M, np.complex128)
        row[:HN] = v1 @ T11 + v2r @ Hh
        row[HN:] = (v1 @ Hh + v2r @ T11)[Jr]
        u1 = V[:HN, 0]; u2r = V[HN:, 0][Jr]
        col = np.empty(M, np.complex128)
        col[:HN] = T11 @ u1 + Hh @ u2r
        col[HN:] = (Hh @ u1 + T11 @ u2r)[Jr]
        # F0' with all diagonal factors + global scale folded
        F0p = F0 * np.outer(q, q) * (Z * ODX * ODY * wl)
        meta.append({
            "w": w,
            "scales": scales,
            "F0p": F0p.astype(np.complex128),
            "corr_row": s * row, "corr_col": s * col,
            "corr_s": s * s * V[0, 0],
        })
    return in_maps, meta


def assemble(results, meta):
    out = np.zeros((1, N_WL, M, M), np.complex64)
    Jr = np.arange(HN)[::-1]
    for md in meta:
        w = md["w"]
        qs = {}
        for ai, a in enumerate(('+', '-')):
            r = results[2 * w + ai]
            b_self, b_oth = a, ('-' if a == '+' else '+')
            for key, ab in (("gs", (a, b_self)), ("go", (a, b_oth))):
                g = r[key].astype(f32)
                qs[ab] = (g[:, :HN].astype(f64) + 1j * g[:, HN:].astype(f64)) \
                    / md["scales"][ab]
        Q1 = qs[('+', '+')]; Q2 = qs[('+', '-')]
        Q3 = qs[('-', '+')]; Q4 = qs[('-', '-')]
        Gf = np.empty((M, M), np.complex128)
        Gf[:HN, :HN] = Q1 + Q2 + Q3 + Q4
        Gf[:HN, HN:] = (Q1 - Q2 + Q3 - Q4)[:, Jr]
        Gf[HN:, :HN] = (Q1 + Q2 - Q3 - Q4)[Jr, :]
        Gf[HN:, HN:] = (Q1 - Q2 - Q3 + Q4)[Jr][:, Jr]
        Gf[M - 1, :] -= md["corr_row"]
        Gf[:, M - 1] -= md["corr_col"]
        Gf[M - 1, M - 1] += md["corr_s"]
        out[0, w] = (md["F0p"] * Gf).astype(np.complex64)
    return out


# ---------------- golden (numpy) model of the device program ----------------

def golden_core(inp):
    def split(x):
        zi = np.vstack([x[:, HN * t:HN * (t + 1)] for t in range(NT)])
        zr = np.vstack([x[:, HN * (NT + t):HN * (NT + t + 1)] for t in range(NT)])
        return zr, zi

    def karatsuba(Ar, Ai, As, Br, Bi, Bs):
        P1 = Ar.astype(f32).T @ Br.astype(f32)
        P2 = Ai.astype(f32).T @ Bi.astype(f32)
        P3 = As.astype(f32).T @ Bs.astype(f32)
        Xr = f16(P1 - P2)
        Xi = f16(P3 - f32(P1 + P2))
        return Xr, Xi

    tsr, tsi = split(inp["ts"]); tss = f16(tsr.astype(f32) + tsi.astype(f32))
    tor_, toi = split(inp["to"]); tos = f16(tor_.astype(f32) + toi.astype(f32))
    out = {}
    for key, vkey, (br, bi, bs) in (("gs", "vs", (tsr, tsi, tss)),
                                    ("go", "vo", (tor_, toi, tos))):
        vr, vi = split(inp[vkey]); vv = f16(vr.astype(f32) + vi.astype(f32))
        Sr, Si = karatsuba(vr, vi, vv, tsr, tsi, tss)
        Ss = f16(Sr.astype(f32) + Si.astype(f32))
        Gr, Gi = karatsuba(Sr, Si, Ss, br, bi, bs)
        out[key] = np.concatenate([Gr, Gi], axis=1)
    return out


def golden(field_real, field_imag, wavelengths):
    in_maps, meta = host_prepare(field_real, field_imag, wavelengths)
    results = [golden_core(m) for m in in_maps]
    return assemble(results, meta)


# ---------------- bass program ----------------

_PROGRAM = None


def build_program():
    import concourse.bass as bass
    import concourse.tile as tile
    import concourse.mybir as mybir
    from concourse import bacc

    dt = mybir.dt
    ALU = mybir.AluOpType

    nc = bacc.Bacc("TRN2", target_bir_lowering=False, debug=False, num_devices=8)

    ts_d = nc.dram_tensor("ts", [P, 4096], dt.float16, kind="ExternalInput").ap()
    to_d = nc.dram_tensor("to", [P, 4096], dt.float16, kind="ExternalInput").ap()
    vs_d = nc.dram_tensor("vs", [P, 4096], dt.float16, kind="ExternalInput").ap()
    vo_d = nc.dram_tensor("vo", [P, 4096], dt.float16, kind="ExternalInput").ap()
    gs_d = nc.dram_tensor("gs", [HN, 1024], dt.float16, kind="ExternalOutput").ap()
    go_d = nc.dram_tensor("go", [HN, 1024], dt.float16, kind="ExternalOutput").ap()

    B2 = NT * HN  # 2048

    with tile.TileContext(nc) as tc:
      with tc.tile_pool(name="persist", bufs=1) as pp, \
           tc.tile_pool(name="psum", bufs=1, space="PSUM") as pspool, \
           tc.tile_pool(name="tmp", bufs=4) as tp:

        def bigtile(nm):
            return pp.tile([P, 4096], dt.float16, tag=nm, name=nm)
        TSa, TOa, VSa, VOa = bigtile("TSa"), bigtile("TOa"), bigtile("VSa"), bigtile("VOa")

        def plane_aps(big):
            iA = [big[:, HN * t:HN * (t + 1)] for t in range(NT)]
            rA = [big[:, B2 + HN * t:B2 + HN * (t + 1)] for t in range(NT)]
            return rA, iA
        TSrA, TSiA = plane_aps(TSa)
        TOrA, TOiA = plane_aps(TOa)
        VSrA, VSiA = plane_aps(VSa)
        VOrA, VOiA = plane_aps(VOa)

        def planes(nm):
            return [pp.tile([P, HN], dt.float16, tag=f"{nm}{t}", name=f"{nm}{t}")
                    for t in range(NT)]
        TSs, TOs, VSs, VOs = planes("TSs"), planes("TOs"), planes("VSs"), planes("VOs")
        Sr = {b: planes(f"S{b}r") for b in "so"}
        Si = {b: planes(f"S{b}i") for b in "so"}
        Ss = {b: planes(f"S{b}s") for b in "so"}

        wlhs = pp.tile([P, P], dt.float16, tag="wlhs", name="wlhs")
        wrhs = pp.tile([P, HN], dt.float16, tag="wrhs", name="wrhs")
        nc.vector.memset(wlhs[:], 0.0)
        nc.vector.memset(wrhs[:], 0.0)

        # ---- input DMAs in consumption order (P2s, P2o, P1s, P1o, P3s/P3o,
        # stage2-other) so matmul passes chase arrivals with no stalls ----
        # sync: tsi kt0-3, tsr kt0-3, vor kt0-3, to halves
        for t in range(NT):
            nc.sync.dma_start(TSiA[t], ts_d[:, HN * t:HN * (t + 1)])
        for t in range(NT):
            nc.sync.dma_start(TSrA[t], ts_d[:, B2 + HN * t:B2 + HN * (t + 1)])
        for t in range(NT):
            nc.sync.dma_start(VOrA[t], vo_d[:, B2 + HN * t:B2 + HN * (t + 1)])
        nc.sync.dma_start(TOa[:, 0:B2], to_d[:, 0:B2])
        nc.sync.dma_start(TOa[:, B2:4096], to_d[:, B2:4096])
        # gpsimd: vsi kt0-3, voi kt0-3, vsr kt0-3
        for t in range(NT):
            nc.gpsimd.dma_start(VSiA[t], vs_d[:, HN * t:HN * (t + 1)])
        for t in range(NT):
            nc.gpsimd.dma_start(VOiA[t], vo_d[:, HN * t:HN * (t + 1)])
        for t in range(NT):
            nc.gpsimd.dma_start(VSrA[t], vs_d[:, B2 + HN * t:B2 + HN * (t + 1)])

        _wn = [0]

        def warmup(n):
            for _ in range(n):
                i = _wn[0]
                _wn[0] += 1
                wp = pspool.tile([P, HN], dt.float32, tag=f"ps{6 + i % 2}", name=f"wps{i}")
                nc.tensor.matmul(wp[:], lhsT=wlhs[:], rhs=wrhs[:], start=True, stop=True)

        warmup(8)

        # sum planes: vector takes ts/vs (needed by P3s ~ mid-flight),
        # gpsimd takes vo/to (needed later)
        for t in range(NT):
            nc.vector.tensor_tensor(out=TSs[t][:], in0=TSrA[t], in1=TSiA[t], op=ALU.add)
            nc.vector.tensor_tensor(out=VSs[t][:], in0=VSrA[t], in1=VSiA[t], op=ALU.add)
        for t in range(NT):
            nc.gpsimd.tensor_tensor(out=VOs[t][:], in0=VOrA[t], in1=VOiA[t], op=ALU.add)
            nc.gpsimd.tensor_tensor(out=TOs[t][:], in0=TOrA[t][:],
                                    in1=TOiA[t][:], op=ALU.add)

        A = lambda ts_: [t[:] for t in ts_]
        TSsA, TOsA, VSsA, VOsA = A(TSs), A(TOs), A(VSs), A(VOs)
        SrA = {b: A(Sr[b]) for b in "so"}
        SiA = {b: A(Si[b]) for b in "so"}
        SsA = {b: A(Ss[b]) for b in "so"}

        # ---- stage 1, both b-blocks, stream-chasing kt-outer passes ----
        # banks: self-set mt -> bank mt, other-set mt -> bank 4+mt
        def acc_pass(ps, lhs, rhs):
            for kt in range(NT):
                st, sp = (kt == 0), (kt == NT - 1)
                for mt in range(NT):
                    msl = slice(P * mt, P * (mt + 1))
                    nc.tensor.matmul(ps[mt][:], lhsT=lhs[kt][:, msl], rhs=rhs[kt],
                                     start=st, stop=sp)

        p2S = [pspool.tile([P, HN], dt.float32, tag=f"ps{mt}", name=f"p2S{mt}")
               for mt in range(NT)]
        p2O = [pspool.tile([P, HN], dt.float32, tag=f"ps{4 + mt}", name=f"p2O{mt}")
               for mt in range(NT)]
        acc_pass(p2S, VSiA, TSiA)
        acc_pass(p2O, VOiA, TSiA)
        p2cS, p2cO = [], []
        for mt in range(NT):
            c = tp.tile([P, HN], dt.float32, tag="p2c", name=f"p2cS{mt}")
            nc.scalar.mul(c[:], p2S[mt][:], 1.0)
            p2cS.append(c)
        for mt in range(NT):
            c = tp.tile([P, HN], dt.float32, tag="p2co", name=f"p2cO{mt}")
            nc.scalar.mul(c[:], p2O[mt][:], 1.0)
            p2cO.append(c)
        p1S = [pspool.tile([P, HN], dt.float32, tag=f"ps{mt}", name=f"p1S{mt}")
               for mt in range(NT)]
        acc_pass(p1S, VSrA, TSrA)
        p1O = [pspool.tile([P, HN], dt.float32, tag=f"ps{4 + mt}", name=f"p1O{mt}")
               for mt in range(NT)]
        acc_pass(p1O, VOrA, TSrA)
        # Xr/t01 for self while P1o runs on the PE
        t01S, t01O = [], []
        for mt in range(NT):
            t01 = tp.tile([P, HN], dt.float32, tag="t01", name=f"t01S{mt}")
            nc.vector.tensor_tensor(out=SrA['s'][mt], in0=p1S[mt][:], in1=p2cS[mt][:],
                                    op=ALU.subtract)
            nc.vector.tensor_tensor(out=t01[:], in0=p1S[mt][:], in1=p2cS[mt][:], op=ALU.add)
            t01S.append(t01)
        # P3 self sweeps (bank mt), with per-mt combines trailing
        for mt in range(NT):
            msl = slice(P * mt, P * (mt + 1))
            p3 = pspool.tile([P, HN], dt.float32, tag=f"ps{mt}", name=f"p3S{mt}")
            for kt in range(NT):
                nc.tensor.matmul(p3[:], lhsT=VSsA[kt][:, msl], rhs=TSsA[kt],
                                 start=(kt == 0), stop=(kt == NT - 1))
            if mt == 0:
                for m2 in range(NT):
                    t01 = tp.tile([P, HN], dt.float32, tag="t01o", name=f"t01O{m2}")
                    nc.vector.tensor_tensor(out=SrA['o'][m2], in0=p1O[m2][:],
                                            in1=p2cO[m2][:], op=ALU.subtract)
                    nc.vector.tensor_tensor(out=t01[:], in0=p1O[m2][:], in1=p2cO[m2][:],
                                            op=ALU.add)
                    t01O.append(t01)
            nc.vector.tensor_tensor(out=SiA['s'][mt], in0=p3[:], in1=t01S[mt][:],
                                    op=ALU.subtract)
            nc.vector.tensor_tensor(out=SsA['s'][mt], in0=SrA['s'][mt], in1=SiA['s'][mt],
                                    op=ALU.add)
        for mt in range(NT):
            msl = slice(P * mt, P * (mt + 1))
            p3 = pspool.tile([P, HN], dt.float32, tag=f"ps{4 + mt}", name=f"p3O{mt}")
            for kt in range(NT):
                nc.tensor.matmul(p3[:], lhsT=VOsA[kt][:, msl], rhs=TSsA[kt],
                                 start=(kt == 0), stop=(kt == NT - 1))
            nc.vector.tensor_tensor(out=SiA['o'][mt], in0=p3[:], in1=t01O[mt][:],
                                    op=ALU.subtract)
            nc.vector.tensor_tensor(out=SsA['o'][mt], in0=SrA['o'][mt], in1=SiA['o'][mt],
                                    op=ALU.add)

        gctr = [0]

        def run_group(phase, mt, out_r, out_i, lhs_parts, rhs_parts,
                      split_tail=False, eager=None):
            g = gctr[0]
            gctr[0] += 1
            b0 = (3 * g) % 8
            lr, li, ls = lhs_parts
            rr, ri, rs = rhs_parts
            msl = slice(P * mt, P * (mt + 1))
            p2 = pspool.tile([P, HN], dt.float32, tag=f"ps{b0}", name=f"p2_{phase}_{mt}")
            p1 = pspool.tile([P, HN], dt.float32, tag=f"ps{(b0 + 1) % 8}", name=f"p1_{phase}_{mt}")
            for kt in range(NT):
                nc.tensor.matmul(p2[:], lhsT=li[kt][:, msl], rhs=ri[kt],
                                 start=(kt == 0), stop=(kt == NT - 1))
            p2c = tp.tile([P, HN], dt.float32, tag="p2c", name=f"p2c_{phase}_{mt}")
            if eager is None:
                nc.scalar.mul(p2c[:], p2[:], 1.0)
            else:
                # tail group: scalar-queue waits get coarsened to the final
                # matmul; vector's deps are precise, so stage psum there.
                nc.vector.tensor_copy(p2c[:], p2[:])
            for kt in range(NT):
                nc.tensor.matmul(p1[:], lhsT=lr[kt][:, msl], rhs=rr[kt],
                                 start=(kt == 0), stop=(kt == NT - 1))
            t01 = tp.tile([P, HN], dt.float32, tag="t01", name=f"t01_{phase}_{mt}")
            nc.vector.tensor_tensor(out=t01[:], in0=p1[:], in1=p2c[:], op=ALU.add)
            nc.vector.tensor_tensor(out=out_r, in0=p1[:], in1=p2c[:], op=ALU.subtract)
            if eager is not None:
                out_d, omsl, gtile = eager
                nc.sync.dma_start(out_d[omsl, 0:HN], gtile[:, 0:HN])
            halves = ((0, HN),) if not split_tail else ((0, HN // 2), (HN // 2, HN))
            for hi, (c0, c1) in enumerate(halves):
                wdt = c1 - c0
                full = (wdt == HN)
                p3 = pspool.tile([P, wdt], dt.float32,
                                 tag=f"ps{(b0 + 2) % 8}" if hi == 0 else f"ps{(b0 + 3) % 8}",
                                 name=f"p3_{phase}_{mt}_{hi}")
                for kt in range(NT):
                    nc.tensor.matmul(p3[:], lhsT=ls[kt][:, msl],
                                     rhs=rs[kt] if full else rs[kt][:, c0:c1],
                                     start=(kt == 0), stop=(kt == NT - 1))
                nc.vector.tensor_tensor(out=out_i if full else out_i[:, c0:c1],
                                        in0=p3[:],
                                        in1=t01[:] if full else t01[:, c0:c1],
                                        op=ALU.subtract)
                if eager is not None:
                    out_d, omsl, gtile = eager
                    nc.sync.dma_start(out_d[omsl, HN + c0:HN + c1],
                                      gtile[:, HN + c0:HN + c1])

        def stage2(phase, b, rhs_parts, out_d, mts):
            for mt in mts:
                gtile = tp.tile([P, 1024], dt.float16, tag="gout", name=f"g_{phase}_{mt}")
                msl = slice(P * mt, P * (mt + 1))
                run_group(phase, mt, gtile[:, 0:HN], gtile[:, HN:1024],
                          (SrA[b], SiA[b], SsA[b]), rhs_parts)
                nc.sync.dma_start(out_d[msl, :], gtile[:])

        stage2(2, 's', (TSrA, TSiA, TSsA), gs_d, range(NT))
        stage2(3, 'o', (TOrA, TOiA, TOsA), go_d, (0, 1))

        # ---- last two groups: P2/P1 sweeps front-loaded with hand-picked
        # early-free banks so the tail is only P3 halves + Gi + DMA ----
        BK = {2: (2, 3, 4, 5), 3: (6, 7, 0, 1)}
        lp2, lp2c, lp1, lt01, lgt = {}, {}, {}, {}, {}
        for mt in (2, 3):
            msl = slice(P * mt, P * (mt + 1))
            lp2[mt] = pspool.tile([P, HN], dt.float32, tag=f"ps{BK[mt][0]}",
                                  name=f"lp2_{mt}")
            for kt in range(NT):
                nc.tensor.matmul(lp2[mt][:], lhsT=SiA['o'][kt][:, msl], rhs=TOiA[kt],
                                 start=(kt == 0), stop=(kt == NT - 1))
            c = tp.tile([P, HN], dt.float32, tag="p2c", name=f"lp2c_{mt}")
            if mt == 2:
                nc.scalar.mul(c[:], lp2[mt][:], 1.0)
            else:
                nc.vector.tensor_copy(c[:], lp2[mt][:])
            lp2c[mt] = c
        for mt in (2, 3):
            msl = slice(P * mt, P * (mt + 1))
            lp1[mt] = pspool.tile([P, HN], dt.float32, tag=f"ps{BK[mt][1]}",
                                  name=f"lp1_{mt}")
            for kt in range(NT):
                nc.tensor.matmul(lp1[mt][:], lhsT=SrA['o'][kt][:, msl], rhs=TOrA[kt],
                                 start=(kt == 0), stop=(kt == NT - 1))
        for mt in (2, 3):
            msl = slice(P * mt, P * (mt + 1))
            gtile = tp.tile([P, 1024], dt.float16, tag="gout", name=f"lg_{mt}")
            t01 = tp.tile([P, HN], dt.float32, tag="t01", name=f"lt01_{mt}")
            nc.vector.tensor_tensor(out=t01[:], in0=lp1[mt][:], in1=lp2c[mt][:],
                                    op=ALU.add)
            nc.vector.tensor_tensor(out=gtile[:, 0:HN], in0=lp1[mt][:],
                                    in1=lp2c[mt][:], op=ALU.subtract)
            nc.sync.dma_start(go_d[msl, 0:HN], gtile[:, 0:HN])
            lt01[mt], lgt[mt] = t01, gtile
        for mt in (2, 3):
            msl = slice(P * mt, P * (mt + 1))
            for hi, (c0, c1) in enumerate(((0, HN // 2), (HN // 2, HN))):
                p3 = pspool.tile([P, HN // 2], dt.float32, tag=f"ps{BK[mt][2 + hi]}",
                                 name=f"lp3_{mt}_{hi}")
                for kt in range(NT):
                    nc.tensor.matmul(p3[:], lhsT=SsA['o'][kt][:, msl],
                                     rhs=TOsA[kt][:, c0:c1],
                                     start=(kt == 0), stop=(kt == NT - 1))
                nc.vector.tensor_tensor(out=lgt[mt][:, HN + c0:HN + c1], in0=p3[:],
                                        in1=lt01[mt][:, c0:c1], op=ALU.subtract)
                eng = nc.sync.dma_start if hi == 0 else nc.scalar.dma_start
                eng(go_d[msl, HN + c0:HN + c1], lgt[mt][:, HN + c0:HN + c1])

    nc.compile()
    return nc


def get_program():
    global _PROGRAM
    if _PROGRAM is None:
        _PROGRAM = build_program()
    return _PROGRAM


def kernel(field_real, field_imag, wavelengths):
    field_real = np.asarray(field_real)
    field_imag = np.asarray(field_imag)
    wavelengths = np.asarray(wavelengths)
    in_maps, meta = host_prepare(field_real, field_imag, wavelengths)
    from concourse.bass_utils import run_bass_kernel_spmd
    nc = get_program()
    res = run_bass_kernel_spmd(nc, in_maps, core_ids=list(range(8)))
    return assemble(res.results, meta)


if __name__ == "__main__":
    import jax
    import reference as ref
    cpu = jax.devices("cpu")[0]
    with jax.default_device(cpu):
        inputs = {k: np.asarray(v) for k, v in ref.setup_inputs().items()}
        expected = np.asarray(ref.reference(**{k: jax.device_put(v, cpu)
                                               for k, v in inputs.items()}))
    got = golden(np.asarray(inputs["field_real"]), np.asarray(inputs["field_imag"]),
                 np.asarray(inputs["wavelengths"]))
    err = np.abs(got - expected)
    print(f"golden absmax err {err.max():.4g} rel {err.max() / np.abs(expected).max():.4g}")


# revision 43
# speedup vs baseline: 1.1411x; 1.0139x over previous
"""Trainium2 Bass kernel for nn_CZT_prop: chirp-z (Bluestein) optical propagation.

Math: per wavelength both CZT axes share the transform M = diag(q) Tc diag(p)
with Tc[c,k] = tau(c-k), tau(d) = exp(-i*aw*d^2/2) an EVEN Toeplitz kernel, so
Tc is centrosymmetric and block-diagonalizes: Tc = K blockdiag(T+, T-) K / 2,
T+/-[c,k] = tau(c-k) +/- tau(c+k-1023), size 512.  The 2D result
    out = F0' . (Tc V Tc^T),   V = (field.F) * outer(p,p)
splits into four half-size quadrant products G_ab = T_a Vt_ab T_b (a,b in {+,-})
where Vt_ab are the +/- fold combos of V (host-prepared).  This HALVES the
device MACs vs the dense formulation.  The recombine (16 flip-adds), the F0'
multiply, and a rank-1 correction for the reference's zero-padded corner entry
[1023,0] of Tc are all host-side assembly.

Device per core (= one (wavelength, a-class)): two chained Karatsuba complex
matmul stages, contraction 512:
    S_b[j,c] = sum_k Vt_ab[k,j] T_a[k,c]     (b in {self, other})
    G_ab[c,d] = sum_j S_b[j,c] T_b[j,d]
192 fp16 matmuls of [128x512]@K=128 = 98304 PE cycles.  Sharding: 8 cores =
4 wavelengths x 2 centro-classes.  Zero communication.
"""
import math
import numpy as np

f32 = np.float32
f16 = np.float16
f64 = np.float64

# ---- static geometry (mirrors the problem spec) ----
H = 1024
M = 1024
N_WL = 4
DX = 100e-6
ODX = 10e-6
ODY = 10e-6
Z = 0.05
TWO_PI = 2.0 * np.pi
P = 128
HN = 512
NT = 4          # 128-row tiles per 512
X_IN = np.linspace(-H * DX / 2, H * DX / 2, H).astype(f64)
X_OUT = np.linspace(-M * ODX / 2, M * ODX / 2, M).astype(f64)


def _pow2_below(x):
    return 2.0 ** math.floor(math.log2(x))


def czt_factors(wl):
    """q[c], p[k], aw for the 1-axis CZT: out[c] = q[c] sum_k tau(c-k) p[k] x[k]."""
    Dm = wl * Z / DX
    f1 = X_OUT[0] + Dm / 2
    f2 = X_OUT[-1] + Dm / 2
    D1 = f1 + (M * Dm + f2 - f1) / (2 * M)
    D2 = f2 + (M * Dm + f2 - f1) / (2 * M)
    alpha_A = TWO_PI * D1 / Dm
    aw = -TWO_PI * (D1 - D2) / (M * Dm)
    k = np.arange(H, dtype=f64)
    c = np.arange(M, dtype=f64)
    h = lambda x: np.exp(1j * aw * x * x / 2)
    l = c / M * (D2 - D1) + D1
    m_shift = np.exp(-1j * TWO_PI * l * (-H / 2 + 0.5) / Dm)
    q = m_shift * h(c) * np.exp(-1j * aw * c) * np.exp(-1j * aw / 2)
    p = np.exp(-1j * alpha_A * k) * h(k) * np.exp(1j * aw * k)
    return q, p, aw


def _rs_kernel_full(xg, wl):
    """RS transfer kernel on the full plane via its 4-fold quad symmetry."""
    kv = TWO_PI / wl
    xh = xg[:HN]
    r2 = xh[:, None] ** 2 + xh[None, :] ** 2 + Z * Z
    r = np.sqrt(r2)
    aa = (Z / TWO_PI) / (r2 * r)
    bt = (kv * Z / TWO_PI) / r2
    ph = kv * r
    cq, sq = np.cos(ph), np.sin(ph)
    Fq = (aa * cq + bt * sq) + 1j * (aa * sq - bt * cq)
    return np.block([[Fq, Fq[:, ::-1]], [Fq[::-1, :], Fq[::-1, ::-1]]])


def host_prepare(field_real, field_imag, wavelengths):
    """Per-core device inputs + assembly metadata.  All f64 host math."""
    wls = np.asarray(wavelengths, f64)
    Jr = np.arange(HN)[::-1]
    in_maps = [None] * 8
    meta = []
    for w in range(N_WL):
        wl = f64(wls[w])
        q, p, aw = czt_factors(wl)
        tau = lambda d: np.exp(-1j * aw * np.asarray(d, f64) ** 2 / 2)
        F = _rs_kernel_full(X_IN, wl)
        F0 = _rs_kernel_full(X_OUT, wl)
        U = (np.asarray(field_real[0, w], f64)
             + 1j * np.asarray(field_imag[0, w], f64)) * F
        V = U * np.outer(p, p)

        cc = np.arange(HN, dtype=f64)[:, None]
        kk = np.arange(HN, dtype=f64)[None, :]
        tnear = tau(cc - kk)
        tfar = tau(cc + kk - (H - 1))
        Tp = tnear + tfar
        Tm = tnear - tfar

        V11 = V[:HN, :HN]; V12 = V[:HN, HN:]
        V21 = V[HN:, :HN]; V22 = V[HN:, HN:]
        A12 = V12[:, Jr]; A21 = V21[Jr, :]; A22 = V22[Jr][:, Jr]
        Vt = {('+', '+'): (V11 + A12 + A21 + A22) / 4,
              ('+', '-'): (V11 - A12 + A21 - A22) / 4,
              ('-', '+'): (V11 + A12 - A21 - A22) / 4,
              ('-', '-'): (V11 - A12 - A21 + A22) / 4}

        # pow2 scaling: one scale per (a,b) block, chosen so BOTH stage
        # outputs stay in fp16 range with ~8x headroom (stage-2 dominates).
        mt2 = {'+': float(np.mean(np.abs(Tp) ** 2)),
               '-': float(np.mean(np.abs(Tm) ** 2))}
        scales = {}
        for ab, Vab in Vt.items():
            fro2 = float(np.sum(np.abs(Vab) ** 2))
            s2_raw = 8.0 * math.sqrt(0.5 * mt2[ab[0]] * mt2[ab[1]] * fro2)
            scales[ab] = _pow2_below(8192.0 / max(s2_raw, 1e-300))

        def detile(z):
            """complex [512,512] -> [128, 4096]: i-plane kt-blocks then r."""
            zi = f16(z.imag); zr = f16(z.real)
            blocks = [zi[P * t:P * (t + 1), :] for t in range(NT)] \
                   + [zr[P * t:P * (t + 1), :] for t in range(NT)]
            return np.ascontiguousarray(np.concatenate(blocks, axis=1))

        Tdev = {'+': detile(Tp), '-': detile(Tm)}
        for ai, a in enumerate(('+', '-')):
            b_self, b_oth = a, ('-' if a == '+' else '+')
            vs = Vt[(a, b_self)] * scales[(a, b_self)]
            vo = Vt[(a, b_oth)] * scales[(a, b_oth)]
            in_maps[2 * w + ai] = {
                "ts": Tdev[a],
                "to": Tdev[b_oth],
                "vs": detile(vs),
                "vo": detile(vo),
            }

        # rank-1 corner correction (f64, exact): out = Tc_ref V Tc_ref^T,
        # Tc_ref = Tc - s e_1023 e_0^T.  Row/col vectors via T+/- blocks:
        # T11 = (Tp+Tm)/2, H = (Tp-Tm)/2 (T12 = H J, T21 = J H, T22 = J T11 J)
        s = tau(f64(H - 1))
        T11 = (Tp + Tm) / 2
        Hh = (Tp - Tm) / 2
        v1 = V[0, :HN]; v2r = V[0, HN:][Jr]
        row = np.empty(M, np.complex128)
        row[:HN] = v1 @ T11 + v2r @ Hh
        row[HN:] = (v1 @ Hh + v2r @ T11)[Jr]
        u1 = V[:HN, 0]; u2r = V[HN:, 0][Jr]
        col = np.empty(M, np.complex128)
        col[:HN] = T11 @ u1 + Hh @ u2r
        col[HN:] = (Hh @ u1 + T11 @ u2r)[Jr]
        # F0' with all diagonal factors + global scale folded
        F0p = F0 * np.outer(q, q) * (Z * ODX * ODY * wl)
        meta.append({
            "w": w,
            "scales": scales,
            "F0p": F0p.astype(np.complex128),
            "corr_row": s * row, "corr_col": s * col,
            "corr_s": s * s * V[0, 0],
        })
    return in_maps, meta


def assemble(results, meta):
    out = np.zeros((1, N_WL, M, M), np.complex64)
    Jr = np.arange(HN)[::-1]
    for md in meta:
        w = md["w"]
        qs = {}
        for ai, a in enumerate(('+', '-')):
            r = results[2 * w + ai]
            b_self, b_oth = a, ('-' if a == '+' else '+')
            for key, ab in (("gs", (a, b_self)), ("go", (a, b_oth))):
                g = r[key].astype(f32)
                qs[ab] = (g[:, :HN].astype(f64) + 1j * g[:, HN:].astype(f64)) \
                    / md["scales"][ab]
        Q1 = qs[('+', '+')]; Q2 = qs[('+', '-')]
        Q3 = qs[('-', '+')]; Q4 = qs[('-', '-')]
        Gf = np.empty((M, M), np.complex128)
        Gf[:HN, :HN] = Q1 + Q2 + Q3 + Q4
        Gf[:HN, HN:] = (Q1 - Q2 + Q3 - Q4)[:, Jr]
        Gf[HN:, :HN] = (Q1 + Q2 - Q3 - Q4)[Jr, :]
        Gf[HN:, HN:] = (Q1 - Q2 - Q3 + Q4)[Jr][:, Jr]
        Gf[M - 1, :] -= md["corr_row"]
        Gf[:, M - 1] -= md["corr_col"]
        Gf[M - 1, M - 1] += md["corr_s"]
        out[0, w] = (md["F0p"] * Gf).astype(np.complex64)
    return out


# ---------------- golden (numpy) model of the device program ----------------

def golden_core(inp):
    def split(x):
        zi = np.vstack([x[:, HN * t:HN * (t + 1)] for t in range(NT)])
        zr = np.vstack([x[:, HN * (NT + t):HN * (NT + t + 1)] for t in range(NT)])
        return zr, zi

    def karatsuba(Ar, Ai, As, Br, Bi, Bs):
        P1 = Ar.astype(f32).T @ Br.astype(f32)
        P2 = Ai.astype(f32).T @ Bi.astype(f32)
        P3 = As.astype(f32).T @ Bs.astype(f32)
        Xr = f16(P1 - P2)
        Xi = f16(P3 - f32(P1 + P2))
        return Xr, Xi

    tsr, tsi = split(inp["ts"]); tss = f16(tsr.astype(f32) + tsi.astype(f32))
    tor_, toi = split(inp["to"]); tos = f16(tor_.astype(f32) + toi.astype(f32))
    out = {}
    for key, vkey, (br, bi, bs) in (("gs", "vs", (tsr, tsi, tss)),
                                    ("go", "vo", (tor_, toi, tos))):
        vr, vi = split(inp[vkey]); vv = f16(vr.astype(f32) + vi.astype(f32))
        Sr, Si = karatsuba(vr, vi, vv, tsr, tsi, tss)
        Ss = f16(Sr.astype(f32) + Si.astype(f32))
        Gr, Gi = karatsuba(Sr, Si, Ss, br, bi, bs)
        out[key] = np.concatenate([Gr, Gi], axis=1)
    return out


def golden(field_real, field_imag, wavelengths):
    in_maps, meta = host_prepare(field_real, field_imag, wavelengths)
    results = [golden_core(m) for m in in_maps]
    return assemble(results, meta)


# ---------------- bass program ----------------

_PROGRAM = None


def build_program():
    import concourse.bass as bass
    import concourse.tile as tile
    import concourse.mybir as mybir
    from concourse import bacc

    dt = mybir.dt
    ALU = mybir.AluOpType

    nc = bacc.Bacc("TRN2", target_bir_lowering=False, debug=False, num_devices=8)

    ts_d = nc.dram_tensor("ts", [P, 4096], dt.float16, kind="ExternalInput").ap()
    to_d = nc.dram_tensor("to", [P, 4096], dt.float16, kind="ExternalInput").ap()
    vs_d = nc.dram_tensor("vs", [P, 4096], dt.float16, kind="ExternalInput").ap()
    vo_d = nc.dram_tensor("vo", [P, 4096], dt.float16, kind="ExternalInput").ap()
    gs_d = nc.dram_tensor("gs", [HN, 1024], dt.float16, kind="ExternalOutput").ap()
    go_d = nc.dram_tensor("go", [HN, 1024], dt.float16, kind="ExternalOutput").ap()

    B2 = NT * HN  # 2048

    with tile.TileContext(nc) as tc:
      with tc.tile_pool(name="persist", bufs=1) as pp, \
           tc.tile_pool(name="psum", bufs=1, space="PSUM") as pspool, \
           tc.tile_pool(name="tmp", bufs=4) as tp:

        def bigtile(nm):
            return pp.tile([P, 4096], dt.float16, tag=nm, name=nm)
        TSa, TOa, VSa, VOa = bigtile("TSa"), bigtile("TOa"), bigtile("VSa"), bigtile("VOa")

        def plane_aps(big):
            iA = [big[:, HN * t:HN * (t + 1)] for t in range(NT)]
            rA = [big[:, B2 + HN * t:B2 + HN * (t + 1)] for t in range(NT)]
            return rA, iA
        TSrA, TSiA = plane_aps(TSa)
        TOrA, TOiA = plane_aps(TOa)
        VSrA, VSiA = plane_aps(VSa)
        VOrA, VOiA = plane_aps(VOa)

        def planes(nm):
            return [pp.tile([P, HN], dt.float16, tag=f"{nm}{t}", name=f"{nm}{t}")
                    for t in range(NT)]
        TSs, TOs, VSs, VOs = planes("TSs"), planes("TOs"), planes("VSs"), planes("VOs")
        Sr = {b: planes(f"S{b}r") for b in "so"}
        Si = {b: planes(f"S{b}i") for b in "so"}
        Ss = {b: planes(f"S{b}s") for b in "so"}

        wlhs = pp.tile([P, P], dt.float16, tag="wlhs", name="wlhs")
        wrhs = pp.tile([P, HN], dt.float16, tag="wrhs", name="wrhs")
        nc.vector.memset(wlhs[:], 0.0)
        nc.vector.memset(wrhs[:], 0.0)

        # ---- input DMAs in consumption order (P2s, P2o, P1s, P1o, P3s/P3o,
        # stage2-other) so matmul passes chase arrivals with no stalls ----
        # sync: tsi kt0-3, tsr kt0-3, vor kt0-3, to halves
        for t in range(NT):
            nc.sync.dma_start(TSiA[t], ts_d[:, HN * t:HN * (t + 1)])
        for t in range(NT):
            nc.sync.dma_start(TSrA[t], ts_d[:, B2 + HN * t:B2 + HN * (t + 1)])
        for t in range(NT):
            nc.sync.dma_start(VOrA[t], vo_d[:, B2 + HN * t:B2 + HN * (t + 1)])
        nc.sync.dma_start(TOa[:, 0:B2], to_d[:, 0:B2])
        nc.sync.dma_start(TOa[:, B2:4096], to_d[:, B2:4096])
        # gpsimd: vsi kt0-3, voi kt0-3, vsr kt0-3
        for t in range(NT):
            nc.gpsimd.dma_start(VSiA[t], vs_d[:, HN * t:HN * (t + 1)])
        for t in range(NT):
            nc.gpsimd.dma_start(VOiA[t], vo_d[:, HN * t:HN * (t + 1)])
        for t in range(NT):
            nc.gpsimd.dma_start(VSrA[t], vs_d[:, B2 + HN * t:B2 + HN * (t + 1)])

        _wn = [0]

        def warmup(n):
            for _ in range(n):
                i = _wn[0]
                _wn[0] += 1
                wp = pspool.tile([P, HN], dt.float32, tag=f"ps{6 + i % 2}", name=f"wps{i}")
                nc.tensor.matmul(wp[:], lhsT=wlhs[:], rhs=wrhs[:], start=True, stop=True)

        warmup(8)

        # sum planes: vector takes ts/vs (needed by P3s ~ mid-flight),
        # gpsimd takes vo/to (needed later)
        for t in range(NT):
            nc.vector.tensor_tensor(out=TSs[t][:], in0=TSrA[t], in1=TSiA[t], op=ALU.add)
            nc.vector.tensor_tensor(out=VSs[t][:], in0=VSrA[t], in1=VSiA[t], op=ALU.add)
        for t in range(NT):
            nc.gpsimd.tensor_tensor(out=VOs[t][:], in0=VOrA[t], in1=VOiA[t], op=ALU.add)
            nc.gpsimd.tensor_tensor(out=TOs[t][:], in0=TOrA[t][:],
                                    in1=TOiA[t][:], op=ALU.add)

        A = lambda ts_: [t[:] for t in ts_]
        TSsA, TOsA, VSsA, VOsA = A(TSs), A(TOs), A(VSs), A(VOs)
        SrA = {b: A(Sr[b]) for b in "so"}
        SiA = {b: A(Si[b]) for b in "so"}
        SsA = {b: A(Ss[b]) for b in "so"}

        # ---- stage 1, both b-blocks, stream-chasing kt-outer passes ----
        # banks: self-set mt -> bank mt, other-set mt -> bank 4+mt
        def acc_pass(ps, lhs, rhs):
            for kt in range(NT):
                st, sp = (kt == 0), (kt == NT - 1)
                for mt in range(NT):
                    msl = slice(P * mt, P * (mt + 1))
                    nc.tensor.matmul(ps[mt][:], lhsT=lhs[kt][:, msl], rhs=rhs[kt],
                                     start=st, stop=sp)

        p2S = [pspool.tile([P, HN], dt.float32, tag=f"ps{mt}", name=f"p2S{mt}")
               for mt in range(NT)]
        p2O = [pspool.tile([P, HN], dt.float32, tag=f"ps{4 + mt}", name=f"p2O{mt}")
               for mt in range(NT)]
        acc_pass(p2S, VSiA, TSiA)
        acc_pass(p2O, VOiA, TSiA)
        p2cS, p2cO = [], []
        for mt in range(NT):
            c = tp.tile([P, HN], dt.float32, tag="p2c", name=f"p2cS{mt}")
            nc.scalar.mul(c[:], p2S[mt][:], 1.0)
            p2cS.append(c)
        for mt in range(NT):
            c = tp.tile([P, HN], dt.float32, tag="p2co", name=f"p2cO{mt}")
            nc.scalar.mul(c[:], p2O[mt][:], 1.0)
            p2cO.append(c)
        p1S = [pspool.tile([P, HN], dt.float32, tag=f"ps{mt}", name=f"p1S{mt}")
               for mt in range(NT)]
        acc_pass(p1S, VSrA, TSrA)
        p1O = [pspool.tile([P, HN], dt.float32, tag=f"ps{4 + mt}", name=f"p1O{mt}")
               for mt in range(NT)]
        acc_pass(p1O, VOrA, TSrA)
        # Xr/t01 for self while P1o runs on the PE
        t01S, t01O = [], []
        for mt in range(NT):
            t01 = tp.tile([P, HN], dt.float32, tag="t01", name=f"t01S{mt}")
            nc.vector.tensor_tensor(out=SrA['s'][mt], in0=p1S[mt][:], in1=p2cS[mt][:],
                                    op=ALU.subtract)
            nc.vector.tensor_tensor(out=t01[:], in0=p1S[mt][:], in1=p2cS[mt][:], op=ALU.add)
            t01S.append(t01)
        # P3 self sweeps (bank mt), with per-mt combines trailing
        for mt in range(NT):
            msl = slice(P * mt, P * (mt + 1))
            p3 = pspool.tile([P, HN], dt.float32, tag=f"ps{mt}", name=f"p3S{mt}")
            for kt in range(NT):
                nc.tensor.matmul(p3[:], lhsT=VSsA[kt][:, msl], rhs=TSsA[kt],
                                 start=(kt == 0), stop=(kt == NT - 1))
            if mt == 0:
                for m2 in range(NT):
                    t01 = tp.tile([P, HN], dt.float32, tag="t01o", name=f"t01O{m2}")
                    nc.vector.tensor_tensor(out=SrA['o'][m2], in0=p1O[m2][:],
                                            in1=p2cO[m2][:], op=ALU.subtract)
                    nc.vector.tensor_tensor(out=t01[:], in0=p1O[m2][:], in1=p2cO[m2][:],
                                            op=ALU.add)
                    t01O.append(t01)
            nc.vector.tensor_tensor(out=SiA['s'][mt], in0=p3[:], in1=t01S[mt][:],
                                    op=ALU.subtract)
            nc.vector.tensor_tensor(out=SsA['s'][mt], in0=SrA['s'][mt], in1=SiA['s'][mt],
                                    op=ALU.add)
        for mt in range(NT):
            msl = slice(P * mt, P * (mt + 1))
            p3 = pspool.tile([P, HN], dt.float32, tag=f"ps{4 + mt}", name=f"p3O{mt}")
            for kt in range(NT):
                nc.tensor.matmul(p3[:], lhsT=VOsA[kt][:, msl], rhs=TSsA[kt],
                                 start=(kt == 0), stop=(kt == NT - 1))
            nc.vector.tensor_tensor(out=SiA['o'][mt], in0=p3[:], in1=t01O[mt][:],
                                    op=ALU.subtract)
            nc.vector.tensor_tensor(out=SsA['o'][mt], in0=SrA['o'][mt], in1=SiA['o'][mt],
                                    op=ALU.add)

        gctr = [0]

        def run_group(phase, mt, out_r, out_i, lhs_parts, rhs_parts,
                      split_tail=False, eager=None):
            g = gctr[0]
            gctr[0] += 1
            b0 = (3 * g) % 8
            lr, li, ls = lhs_parts
            rr, ri, rs = rhs_parts
            msl = slice(P * mt, P * (mt + 1))
            p2 = pspool.tile([P, HN], dt.float32, tag=f"ps{b0}", name=f"p2_{phase}_{mt}")
            p1 = pspool.tile([P, HN], dt.float32, tag=f"ps{(b0 + 1) % 8}", name=f"p1_{phase}_{mt}")
            for kt in range(NT):
                nc.tensor.matmul(p2[:], lhsT=li[kt][:, msl], rhs=ri[kt],
                                 start=(kt == 0), stop=(kt == NT - 1))
            p2c = tp.tile([P, HN], dt.float32, tag="p2c", name=f"p2c_{phase}_{mt}")
            if eager is None:
                nc.scalar.mul(p2c[:], p2[:], 1.0)
            else:
                # tail group: scalar-queue waits get coarsened to the final
                # matmul; vector's deps are precise, so stage psum there.
                nc.vector.tensor_copy(p2c[:], p2[:])
            for kt in range(NT):
                nc.tensor.matmul(p1[:], lhsT=lr[kt][:, msl], rhs=rr[kt],
                                 start=(kt == 0), stop=(kt == NT - 1))
            t01 = tp.tile([P, HN], dt.float32, tag="t01", name=f"t01_{phase}_{mt}")
            nc.vector.tensor_tensor(out=t01[:], in0=p1[:], in1=p2c[:], op=ALU.add)
            nc.vector.tensor_tensor(out=out_r, in0=p1[:], in1=p2c[:], op=ALU.subtract)
            if eager is not None:
                out_d, omsl, gtile = eager
                nc.sync.dma_start(out_d[omsl, 0:HN], gtile[:, 0:HN])
            halves = ((0, HN),) if not split_tail else ((0, HN // 2), (HN // 2, HN))
            for hi, (c0, c1) in enumerate(halves):
                wdt = c1 - c0
                full = (wdt == HN)
                p3 = pspool.tile([P, wdt], dt.float32,
                                 tag=f"ps{(b0 + 2) % 8}" if hi == 0 else f"ps{(b0 + 3) % 8}",
                                 name=f"p3_{phase}_{mt}_{hi}")
                for kt in range(NT):
                    nc.tensor.matmul(p3[:], lhsT=ls[kt][:, msl],
                                     rhs=rs[kt] if full else rs[kt][:, c0:c1],
                                     start=(kt == 0), stop=(kt == NT - 1))
                nc.vector.tensor_tensor(out=out_i if full else out_i[:, c0:c1],
                                        in0=p3[:],
                                        in1=t01[:] if full else t01[:, c0:c1],
                                        op=ALU.subtract)
                if eager is not None:
                    out_d, omsl, gtile = eager
                    nc.sync.dma_start(out_d[omsl, HN + c0:HN + c1],
                                      gtile[:, HN + c0:HN + c1])

        def stage2(phase, b, rhs_parts, out_d, mts):
            for mt in mts:
                gtile = tp.tile([P, 1024], dt.float16, tag="gout", name=f"g_{phase}_{mt}")
                msl = slice(P * mt, P * (mt + 1))
                run_group(phase, mt, gtile[:, 0:HN], gtile[:, HN:1024],
                          (SrA[b], SiA[b], SsA[b]), rhs_parts)
                nc.sync.dma_start(out_d[msl, :], gtile[:])

        stage2(2, 's', (TSrA, TSiA, TSsA), gs_d, range(NT))
        stage2(3, 'o', (TOrA, TOiA, TOsA), go_d, (0, 1))

        # ---- last two groups: P2/P1 sweeps front-loaded with hand-picked
        # early-free banks so the tail is only P3 halves + Gi + DMA ----
        BK = {2: (2, 3, 4, 5), 3: (6, 7, 0, 1)}
        lp2, lp2c, lp1, lt01, lgt = {}, {}, {}, {}, {}
        for mt in (2, 3):
            msl = slice(P * mt, P * (mt + 1))
            lp2[mt] = pspool.tile([P, HN], dt.float32, tag=f"ps{BK[mt][0]}",
                                  name=f"lp2_{mt}")
            for kt in range(NT):
                nc.tensor.matmul(lp2[mt][:], lhsT=SiA['o'][kt][:, msl], rhs=TOiA[kt],
                                 start=(kt == 0), stop=(kt == NT - 1))
            c = tp.tile([P, HN], dt.float32, tag="p2c", name=f"lp2c_{mt}")
            # front-loaded P2 executes mid-stream, so scalar's wait threshold
            # is precise here (no final-matmul coarsening)
            nc.scalar.mul(c[:], lp2[mt][:], 1.0)
            lp2c[mt] = c
        for mt in (2, 3):
            msl = slice(P * mt, P * (mt + 1))
            lp1[mt] = pspool.tile([P, HN], dt.float32, tag=f"ps{BK[mt][1]}",
                                  name=f"lp1_{mt}")
            for kt in range(NT):
                nc.tensor.matmul(lp1[mt][:], lhsT=SrA['o'][kt][:, msl], rhs=TOrA[kt],
                                 start=(kt == 0), stop=(kt == NT - 1))
        for mt in (2, 3):
            msl = slice(P * mt, P * (mt + 1))
            gtile = tp.tile([P, 1024], dt.float16, tag="gout", name=f"lg_{mt}")
            t01 = tp.tile([P, HN], dt.float32, tag="t01", name=f"lt01_{mt}")
            nc.vector.tensor_tensor(out=t01[:], in0=lp1[mt][:], in1=lp2c[mt][:],
                                    op=ALU.add)
            nc.vector.tensor_tensor(out=gtile[:, 0:HN], in0=lp1[mt][:],
                                    in1=lp2c[mt][:], op=ALU.subtract)
            nc.sync.dma_start(go_d[msl, 0:HN], gtile[:, 0:HN])
            lt01[mt], lgt[mt] = t01, gtile
        for mt in (2, 3):
            msl = slice(P * mt, P * (mt + 1))
            for hi, (c0, c1) in enumerate(((0, HN // 2), (HN // 2, HN))):
                p3 = pspool.tile([P, HN // 2], dt.float32, tag=f"ps{BK[mt][2 + hi]}",
                                 name=f"lp3_{mt}_{hi}")
                for kt in range(NT):
                    nc.tensor.matmul(p3[:], lhsT=SsA['o'][kt][:, msl],
                                     rhs=TOsA[kt][:, c0:c1],
                                     start=(kt == 0), stop=(kt == NT - 1))
                nc.vector.tensor_tensor(out=lgt[mt][:, HN + c0:HN + c1], in0=p3[:],
                                        in1=lt01[mt][:, c0:c1], op=ALU.subtract)
                eng = nc.sync.dma_start if hi == 0 else nc.scalar.dma_start
                eng(go_d[msl, HN + c0:HN + c1], lgt[mt][:, HN + c0:HN + c1])

    nc.compile()
    return nc


def get_program():
    global _PROGRAM
    if _PROGRAM is None:
        _PROGRAM = build_program()
    return _PROGRAM


def kernel(field_real, field_imag, wavelengths):
    field_real = np.asarray(field_real)
    field_imag = np.asarray(field_imag)
    wavelengths = np.asarray(wavelengths)
    in_maps, meta = host_prepare(field_real, field_imag, wavelengths)
    from concourse.bass_utils import run_bass_kernel_spmd
    nc = get_program()
    res = run_bass_kernel_spmd(nc, in_maps, core_ids=list(range(8)))
    return assemble(res.results, meta)


if __name__ == "__main__":
    import jax
    import reference as ref
    cpu = jax.devices("cpu")[0]
    with jax.default_device(cpu):
        inputs = {k: np.asarray(v) for k, v in ref.setup_inputs().items()}
        expected = np.asarray(ref.reference(**{k: jax.device_put(v, cpu)
                                               for k, v in inputs.items()}))
    got = golden(np.asarray(inputs["field_real"]), np.asarray(inputs["field_imag"]),
                 np.asarray(inputs["wavelengths"]))
    err = np.abs(got - expected)
    print(f"golden absmax err {err.max():.4g} rel {err.max() / np.abs(expected).max():.4g}")
